# revision 62
# baseline (speedup 1.0000x reference)
"""Trainium2 Bass kernel for nn_MultiHeadedAttention_51737176047655.

Multi-head attention with Music-Transformer relative position bias
(skew trick), B=4, L=1024, D=1024, 16 heads, head_dim=64.

Sharding (8 cores): core = 2*b + hg  -> batch b in [0,4), head-group hg in
[0,2).  Each core computes 8 heads for one batch over the full sequence:
  - Wq/Wk/Wv column-sharded [1024, 512], Wo row-sharded [512, 1024]
  - per-core output is a partial [1024, 1024]; host sums the two
    head-group partials per batch (standard TP unshard) and adds bo.

Device algorithm per core (matmuls bf16 in / f32 PSUM accumulate):
  qT/kT/vT arrive host-transposed [d, l]; projections give qhT/khT
  [d', l] (transposed) and vh [l, d'] (natural, with a ones column per
  head for softmax sums).  QE = qh e^T is computed per head (only the
  m < l0+128 span that survives the tri mask), masked via a shifted-tri
  "slab" multiply, and written to a DRAM scratch in the padded layout
  (row stride 1025); reading rows back with stride 1024 materializes
  the skewed Srel exactly (the reference's pad+reshape trick).  scores
  are computed TRANSPOSED (scores^T = kh qh^T, head pairs packed into
  PE row groups via tile_position) and Srel^T is accumulated into the
  same PSUM bank by transpose-by-identity matmuls, skipping
  statically-zero 128x128 blocks; exp via ScalarE (scale=1/8) ->
  unnormalized attn^T (bf16); ctx^T_aug = [vh|1]^T attn^T per head
  (row 64 = softmax denominators Z); 1/Z via a single ScalarE
  Reciprocal activation straight off the PSUM Z row, broadcast across
  64 partitions with a K=1 PE matmul, and applied by the DVE while
  packing ctx^T head-pairs; out = ctx Wo, emitted bf16 (host
  accumulates in f32).
  The TensorE instruction stream is interleaved at j-tile granularity
  (scores of head h / attnV of head h-1 / QE of head h+2, and attnV of
  the last head with the first half of the output projection) so the
  in-order PE never stalls on the exp/normalize pipelines; this keeps
  the PE clock at its top p-state.  DMA count is minimized: batched
  input loads, one sliding-window batched read for the low-half Srel
  stripes, persistent pre-zeroed stripe staging tiles.
No max-subtraction in softmax: logits are ~N(0, 1.4^2), far inside
fp32/exp range (validated vs reference at ~1e-6 in fp32 emulation).
"""

import math
import sys

import numpy as np

sys.path.insert(0, "/opt/trn_rl_repo")

import ml_dtypes  # noqa: E402

BF16 = ml_dtypes.bfloat16

# Problem constants (hardcoded per contract)
B = 4
L = 1024
D = 1024
H = 16
HD = 64
H_LOC = 8  # heads per core
DG = 512  # d' columns per core (H_LOC * HD)
NCORES = 8
MAX_SEQ = 2048
PAD = L + 1  # 1025, padded row stride of the skew scratch
FLAT = L * PAD  # 1049600 elements per head scratch

NLT = L // 128  # 8 l-tiles
NDT = D // 128  # 8 contraction tiles
NPAIR = H_LOC // 2  # 4 head pairs


def _build_bass():
    """Build the single-core SPMD Bass program (same program, per-core data)."""
    import concourse.bass as bass
    import concourse.tile as tile
    from concourse import bacc, mybir
    from concourse.dve_ops import (
        RECIP_APPROX_FAST_CONSTS,
        RECIPROCAL_APPROX_FAST,
    )

    f32 = mybir.dt.float32
    bf16 = mybir.dt.bfloat16
    Exp = mybir.ActivationFunctionType.Exp
    mult = mybir.AluOpType.mult

    nc = bacc.Bacc(
        "TRN2", target_bir_lowering=False, debug=False, enable_asserts=False
    )

    # ---- kernel I/O; qT/kT/vT are host-transposed [d, l] and then
    # host-shuffled to [p, dt, l] so device loads are contiguous ----
    qT_d = nc.declare_dram_parameter("qT", [128, NDT * L], bf16, isOutput=False)
    kT_d = nc.declare_dram_parameter("kT", [128, NDT * L], bf16, isOutput=False)
    vT_d = nc.declare_dram_parameter("vT", [128, NDT * L], bf16, isOutput=False)
    wq_d = nc.declare_dram_parameter("wq", [128, NDT * DG], bf16, isOutput=False)
    wk_d = nc.declare_dram_parameter("wk", [128, NDT * DG], bf16, isOutput=False)
    wv_d = nc.declare_dram_parameter("wv", [128, NDT * DG], bf16, isOutput=False)
    wo_d = nc.declare_dram_parameter("wo", [128, NPAIR * D], bf16, isOutput=False)
    e2_d = nc.declare_dram_parameter("e2", [128, L], bf16, isOutput=False)
    slab_d = nc.declare_dram_parameter("slab", [128, 640], bf16, isOutput=False)
    out_d = nc.declare_dram_parameter("out", [L, D], bf16, isOutput=True)

    # skew scratch, one padded buffer per local head
    scratch = [nc.dram_tensor(f"skew{h}", [FLAT], bf16) for h in range(H_LOC)]

    # block (lt, jt) of Srel is identically zero unless piece A
    # (j <= 2l-1023) or piece B (l+2 <= j <= 2l+3) intersects it.
    def srel_block_nonzero(lt, jt):
        l1 = 128 * lt + 127
        j0, j1 = 128 * jt, 128 * jt + 127
        a = 2 * l1 - 1023 >= j0
        b = (j1 >= 128 * lt + 2) and (j0 <= 2 * l1 + 3)
        return a or b

    def srel_block_lspan(lt, jt):
        """Block-relative [lo, hi) of l rows where Srel(block) is nonzero."""
        l0, l1 = 128 * lt, 128 * lt + 127
        j0, j1 = 128 * jt, 128 * jt + 127
        spans = []
        bs, be = max(l0, -(-(j0 - 3) // 2)), min(l1, j1 - 2)
        if bs <= be:
            spans.append((bs, be))
        as_, ae = max(l0, -(-(j0 + 1023) // 2)), l1
        if as_ <= ae:
            spans.append((as_, ae))
        if not spans:
            return None
        lo = min(s for s, _ in spans) - l0
        hi = max(e for _, e in spans) - l0 + 1
        return lo, hi

    with tile.TileContext(nc) as tc:
        from contextlib import ExitStack

        with ExitStack() as outer:
            # ---------------- persistent pools ----------------
            persist = outer.enter_context(tc.tile_pool(name="persist", bufs=1))
            # projection outputs (live through whole kernel)
            qhT = persist.tile([128, NPAIR, L], bf16)  # [part, pair, l]
            khT = persist.tile([128, NPAIR, L], bf16)
            # vh with ones column per head: [part(j%128), jt, head, 65]
            vh = persist.tile([128, NLT, H_LOC, HD + 1], bf16)
            e2_sb = persist.tile([128, L], bf16)
            slab_sb = persist.tile([128, 640], bf16)
            ctxp = persist.tile([128, NPAIR, L], bf16)  # packed ctx^T per pair
            # all-ones stationary for the K=1 1/Z PE broadcast
            ones1 = persist.tile([1, 64], bf16, name="ones1")
            # persistent stripe staging tiles, pre-zeroed once; data spans are
            # rewritten per head, zero-col/tail spans stay zero across reuse
            stripes = [
                [persist.tile([128, 4, PAD], bf16, name=f"st{lh}{par}")
                 for par in range(2)]
                for lh in range(2)
            ]

            nc.vector.memset(vh[:, :, :, HD : HD + 1], 1.0)
            nc.vector.memset(ones1, 1.0)
            for lh in range(2):
                for par in range(2):
                    nc.gpsimd.memset(stripes[lh][par], 0.0)

            # ---------------- phase 1+2: loads + projections ----
            with ExitStack() as outer2:
                sc_ps = outer2.enter_context(
                    tc.tile_pool(name="sc_ps", bufs=5, space="PSUM")
                )
                qe_ps = sc_ps  # QE shares the scores PSUM slots (tag "sc")
                ctx_ps = None  # opened after mm_ps closes (PSUM bank budget)
                attT = outer2.enter_context(tc.tile_pool(name="attT", bufs=4))
                srl = outer2.enter_context(tc.tile_pool(name="srl", bufs=3))
                zp = outer2.enter_context(tc.tile_pool(name="zp", bufs=2))

                ident = persist.tile([128, 128], bf16, name="ident")
                from concourse.masks import make_identity

                make_identity(nc, ident)

                # short-lived input pools opened last (LIFO close order)
                tin_blk = ExitStack()
                tin = tin_blk.enter_context(tc.tile_pool(name="tin", bufs=1))
                mm_ps = tin_blk.enter_context(
                    tc.tile_pool(name="mm_ps", bufs=3, space="PSUM")
                )
                # NOTE: phase-1 PSUM = mm(3) + sc(5) = 8 banks

                # qT and vT share one slot (vT loads after q-proj drains qT)
                qT = tin.tile([128, NDT, L], bf16, name="qT", tag="xqv")
                kT = tin.tile([128, NDT, L], bf16, name="kT")
                vT = tin.tile([128, NDT, L], bf16, name="vT", tag="xqv")
                wq_sb = tin.tile([128, NDT, DG], bf16, name="wq")
                wk_sb = tin.tile([128, NDT, DG], bf16, name="wk")
                wv_sb = tin.tile([128, NDT, DG], bf16, name="wv")

                def load_xT(dst, src_d, eng):
                    # 2-tile chunks, contiguous per partition in DRAM,
                    # incremental tile availability
                    for c in range(0, NDT, 2):
                        eng.dma_start(
                            out=dst[:, c : c + 2, :],
                            in_=src_d[:, c * L : (c + 2) * L],
                        )

                def load_w(dst, src_d, eng):
                    for c in range(0, NDT, 2):
                        eng.dma_start(
                            out=dst[:, c : c + 2, :],
                            in_=src_d[:, c * DG : (c + 2) * DG],
                        )

                # q + Wq first so the q projections start early; the input
                # wire is ~160GB/s regardless of queue count, so keep all
                # loads ordered on the sync queue
                load_w(wq_sb, wq_d, nc.sync)
                load_xT(qT, qT_d, nc.sync)
                nc.sync.dma_start(out=e2_sb, in_=e2_d[:, :])
                nc.sync.dma_start(out=slab_sb, in_=slab_d[:, :])
                load_w(wk_sb, wk_d, nc.sync)
                load_xT(kT, kT_d, nc.sync)
                load_w(wv_sb, wv_d, nc.sync)

                def proj_pair(w_sb, xT, dst, p):
                    for lh in range(2):
                        ps = mm_ps.tile([128, 512], f32, name="proj_ps", tag="mm")
                        lsl = slice(512 * lh, 512 * (lh + 1))
                        for dt in range(NDT):
                            nc.tensor.matmul(
                                ps,
                                w_sb[:, dt, 128 * p : 128 * (p + 1)],
                                xT[:, dt, lsl],
                                start=(dt == 0),
                                stop=(dt == NDT - 1),
                            )
                        nc.scalar.copy(dst[:, p, lsl], ps)

                def vh_tile(jt):
                    ps = mm_ps.tile([128, 512], f32, name="vh_ps", tag="mm")
                    jsl = slice(128 * jt, 128 * (jt + 1))
                    for dt in range(NDT):
                        nc.tensor.matmul(
                            ps,
                            vT[:, dt, jsl],
                            wv_sb[:, dt, :],
                            start=(dt == 0),
                            stop=(dt == NDT - 1),
                        )
                    # scatter 512 d' columns into per-head [64] slots with one
                    # strided copy (dst strides over the 65-wide head slots)
                    pv = ps[0:128, :]
                    ps3 = bass.AP(
                        tensor=pv.tensor,
                        offset=pv.offset,
                        ap=[list(pv.ap)[0], [HD, H_LOC], [1, HD]],
                    )
                    nc.scalar.copy(vh[:, jt, :, 0:HD], ps3)

                def qe_lt_mm(h, lt):
                    """QE matmuls (half-array config) for row-block lt;
                    returns psum tiles for qe_lt_fix."""
                    p, hl = divmod(h, 2)
                    rows = slice(64 * hl, 64 * (hl + 1))
                    tp = (64 * hl, 0)
                    l0 = 128 * lt
                    lsl = slice(l0, l0 + 128)
                    if lt <= 3:
                        n0 = l0 + 128
                        psm = qe_ps.tile([128, 512], f32, name="qe", tag="sc")
                        nc.tensor.matmul(
                            psm[:, 0:n0],
                            qhT[rows, p, lsl],
                            e2_sb[rows, 0:n0],
                            start=True,
                            stop=True,
                            tile_position=tp,
                        )
                        return (psm, None)
                    psm = qe_ps.tile([128, 512], f32, name="qe", tag="sc")
                    nc.tensor.matmul(
                        psm,
                        qhT[rows, p, lsl],
                        e2_sb[rows, 0:512],
                        start=True,
                        stop=True,
                        tile_position=tp,
                    )
                    n1 = l0 + 128 - 512
                    psm2 = qe_ps.tile([128, 512], f32, name="qe", tag="sc")
                    nc.tensor.matmul(
                        psm2[:, 0:n1],
                        qhT[rows, p, lsl],
                        e2_sb[rows, 512 : 512 + n1],
                        start=True,
                        stop=True,
                        tile_position=tp,
                    )
                    return (psm, psm2)

                def qe_lt_fix(h, lt, pss):
                    """DVE masking + stripe staging + (on the 4th block of an
                    l-half) the stripe DMA."""
                    psm, psm2 = pss
                    lh, a = divmod(lt, 4)
                    big = stripes[lh][h % 2]
                    l0 = 128 * lt
                    stripe = big[:, a, :]
                    if lt <= 3:
                        n0 = l0 + 128
                        nc.vector.tensor_tensor(
                            stripe[:, 1 : 1 + n0],
                            psm[:, 0:n0],
                            slab_sb[:, 512 - l0 : 640],
                            mult,
                        )
                    else:
                        n1 = l0 + 128 - 512
                        if lt == 4:
                            nc.vector.tensor_tensor(
                                stripe[:, 1:513],
                                psm,
                                slab_sb[:, 0:512],
                                mult,
                            )
                        else:
                            # m < 512 is fully below the diagonal: copy
                            nc.vector.tensor_copy(stripe[:, 1:513], psm)
                        nc.vector.tensor_tensor(
                            stripe[:, 513 : 1 + l0 + 128],
                            psm2[:, 0:n1],
                            slab_sb[:, 1024 - l0 : 640],
                            mult,
                        )
                    if a == 3:
                        # one DMA for the 4 padded stripes of this l-half
                        dst = bass.AP(
                            tensor=scratch[h],
                            offset=512 * lh * PAD,
                            ap=[[PAD, 128], [128 * PAD, 4], [1, PAD]],
                        )
                        nc.sync.dma_start(out=dst, in_=big)

                def qe_lt(h, lt):
                    qe_lt_fix(h, lt, qe_lt_mm(h, lt))

                def srel_load(h, lh):
                    if lh == 0:
                        # low l-half: sliding 640-wide j-window per lt
                        # (window start 128*lt covers all nonzero blocks)
                        srel = srl.tile([128, 4, 640], bf16, name="srel")
                        src = bass.AP(
                            tensor=scratch[h],
                            offset=L,
                            ap=[[L, 128], [128 * L + 128, 4], [1, 640]],
                        )
                    else:
                        # high l-half: dense
                        srel = srl.tile([128, 4, L], bf16, name="srel")
                        src = bass.AP(
                            tensor=scratch[h],
                            offset=(512 * lh + 1) * L,
                            ap=[[L, 128], [128 * L, 4], [1, L]],
                        )
                    nc.sync.dma_start(out=srel, in_=src)
                    return srel

                def scores_mm(h, lh, jt):
                    """scores^T matmul (half-array config); returns psum."""
                    p, hl = divmod(h, 2)
                    rows = slice(64 * hl, 64 * (hl + 1))
                    tp = (64 * hl, 0)
                    lsl = slice(512 * lh, 512 * (lh + 1))
                    jsl = slice(128 * jt, 128 * (jt + 1))
                    ps = sc_ps.tile([128, 512], f32, name="sc", tag="sc")
                    nzs = [
                        a for a in range(4)
                        if srel_block_nonzero(4 * lh + a, jt)
                    ]
                    nc.tensor.matmul(
                        ps,
                        khT[rows, p, jsl],
                        qhT[rows, p, lsl],
                        start=True,
                        stop=(len(nzs) == 0),
                        tile_position=tp,
                    )
                    return ps

                def scores_fix(h, lh, jt, ps, srel, att):
                    """Srel^T transpose-adds (full-array config) + exp."""
                    jsl = slice(128 * jt, 128 * (jt + 1))
                    nzs = [
                        a for a in range(4)
                        if srel_block_nonzero(4 * lh + a, jt)
                    ]
                    for i, a in enumerate(nzs):
                        if lh == 0:
                            jr = slice(128 * (jt - a), 128 * (jt - a) + 128)
                            chunk = srel[:, a, jr]
                        else:
                            chunk = srel[:, a, jsl]
                        nc.tensor.matmul(
                            ps[:, 128 * a : 128 * a + 128],
                            chunk,
                            ident,
                            start=False,
                            stop=(i == len(nzs) - 1),
                        )
                    nc.scalar.activation(att[:, jt, :], ps, Exp, scale=0.125)

                def attnv_part(h, halves, lh, jt, cps):
                    nc.tensor.matmul(
                        cps[0 : HD + 1, :],
                        vh[:, jt, h, :],
                        halves[lh][:, jt, :],
                        start=(jt == 0),
                        stop=(jt == NLT - 1),
                    )

                def attnv_finish_a(cps0, zpack):
                    # stage the lh=0 Z row; the real finish happens in _b
                    nc.vector.tensor_copy(zpack[0:1, :], cps0[HD : HD + 1, :])

                def attnv_finish_b(h, cps_both, zpack):
                    p, hl = divmod(h, 2)
                    rows = slice(64 * hl, 64 * (hl + 1))
                    nc.vector.tensor_copy(
                        zpack[32:33, :], cps_both[1][HD : HD + 1, :]
                    )
                    # one fast approximate DVE reciprocal (~51 ULP, 5x faster
                    # than InstReciprocal) covers both Z rows (0 and 32; the
                    # rows between hold garbage and are never read)
                    zinv = zp.tile([33, 512], f32, name="zinv")
                    nc.vector._custom_dve(
                        RECIPROCAL_APPROX_FAST,
                        out=zinv,
                        in0=zpack,
                        **RECIP_APPROX_FAST_CONSTS,
                    )
                    zinvb = [
                        zp.tile([1, 512], bf16, name="zinvb") for _ in range(2)
                    ]
                    for lh in range(2):
                        nc.vector.tensor_copy(
                            zinvb[lh], zinv[32 * lh : 32 * lh + 1, :]
                        )
                    for lh in range(2):
                        # broadcast 1/Z across 64 partitions on the (idle)
                        # GPSIMD engine -- no PSUM, no PE involvement
                        zbs = zp.tile([64, 512], bf16, name="zbs")
                        nc.gpsimd.partition_broadcast(zbs, zinvb[lh])
                        # normalize + pack into head-pair ctx^T (bf16)
                        nc.vector.tensor_tensor(
                            ctxp[rows, p, 512 * lh : 512 * (lh + 1)],
                            cps_both[lh][0:HD, :],
                            zbs,
                            mult,
                        )

                # ---- emission: projections first ----
                for p in range(NPAIR):
                    proj_pair(wq_sb, qT, qhT, p)
                # vT reuses qT's slot; its DMA fires once q-proj drains qT
                load_xT(vT, vT_d, nc.sync)
                for lt in range(8):
                    qe_lt(0, lt)
                for lt in range(8):
                    qe_lt(1, lt)
                for p in range(NPAIR):
                    proj_pair(wk_sb, kT, khT, p)
                for jt in range(NLT):
                    vh_tile(jt)
                tin_blk.close()
                ctx_ps = outer2.enter_context(
                    tc.tile_pool(name="ctx_ps", bufs=3, space="PSUM")
                )
                # wo lives in the space vacated by the input tiles; loaded
                # here (well before the output projection)
                wop = outer2.enter_context(tc.tile_pool(name="wop", bufs=1))
                wo_sb = wop.tile([128, NPAIR, D], bf16, name="wo")
                nc.sync.dma_start(out=wo_sb, in_=wo_d[:, :])
                ost = outer2.enter_context(tc.tile_pool(name="ost", bufs=4))

                # ---- main pipeline: scores(h) / attnV(h-1) / QE(h+2)
                # interleaved at j-tile granularity so the in-order PE
                # always has a ready instruction ----
                pend = None
                fin = None  # deferred finish_b of head h-2
                srel_cur = srel_load(0, 0)
                for h in range(H_LOC):
                    att0 = attT.tile([128, NLT, 512], bf16, name="attnT")
                    att1 = attT.tile([128, NLT, 512], bf16, name="attnT")
                    # prefetch this head's high-half Srel during the low half
                    srel_nxt = srel_load(h, 1)
                    cps_prev = {}
                    zpack = zp.tile([33, 512], f32, name="zpack")
                    for lh in range(2):
                        att = att0 if lh == 0 else att1
                        if lh == 1:
                            srel_cur = srel_nxt
                            if h + 1 < H_LOC:
                                # prefetch the next head's low half
                                srel_nxt = srel_load(h + 1, 0)
                        if pend is not None:
                            hp, halves = pend
                            cps_prev[lh] = ctx_ps.tile(
                                [128, 512], f32, name="cps", tag="cps"
                            )
                        # 2-j-tile batches grouped by PE row-config:
                        # [scores mms + QE mms] (half-array) ->
                        # [transpose-adds + attnV parts] (full-array);
                        # config switches cost ~100ns each on the PE
                        for j0 in range(0, NLT, 2):
                            qe_due = h + 2 < H_LOC
                            lt = 4 * lh + j0 // 2
                            ps_a = scores_mm(h, lh, j0)
                            ps_b = scores_mm(h, lh, j0 + 1)
                            if qe_due:
                                pss = qe_lt_mm(h + 2, lt)
                            scores_fix(h, lh, j0, ps_a, srel_cur, att)
                            scores_fix(h, lh, j0 + 1, ps_b, srel_cur, att)
                            if pend is not None:
                                attnv_part(hp, halves, lh, j0, cps_prev[lh])
                                attnv_part(
                                    hp, halves, lh, j0 + 1, cps_prev[lh]
                                )
                            if qe_due:
                                qe_lt_fix(h + 2, lt, pss)
                            if lh == 0 and j0 == 2 and fin is not None:
                                attnv_finish_b(*fin)
                                fin = None
                        if lh == 0 and pend is not None:
                            attnv_finish_a(cps_prev[0], zpack)
                    if pend is not None:
                        # defer the finish (normalize chain) into the next
                        # head's stream so the head boundary never stalls
                        fin = (hp, cps_prev, zpack)
                    pend = (h, [att0, att1])
                    srel_cur = srel_nxt
                if fin is not None:
                    attnv_finish_b(*fin)
                    fin = None

                # ---- tail: attnV of the last head, interleaved with the
                # first half of the output projection ----
                def outproj_unit(lt, jh, o):
                    lsl = slice(128 * lt, 128 * (lt + 1))
                    jsl = slice(512 * jh, 512 * (jh + 1))
                    ps = sc_ps.tile([128, 512], f32, name="op", tag="sc")
                    for p in range(NPAIR):
                        nc.tensor.matmul(
                            ps,
                            ctxp[:, p, lsl],
                            wo_sb[:, p, jsl],
                            start=(p == 0),
                            stop=(p == NPAIR - 1),
                        )
                    nc.scalar.copy(o[:, jsl], ps)

                def attnv_finish_lh(h, lh, cps):
                    # per-half finish used in the tail (DVE has slack there)
                    p, hl = divmod(h, 2)
                    rows = slice(64 * hl, 64 * (hl + 1))
                    zs = zp.tile([1, 512], f32, name="zs")
                    nc.vector.tensor_copy(zs, cps[HD : HD + 1, :])
                    zi = zp.tile([1, 512], f32, name="zi")
                    nc.vector._custom_dve(
                        RECIPROCAL_APPROX_FAST,
                        out=zi,
                        in0=zs,
                        **RECIP_APPROX_FAST_CONSTS,
                    )
                    zib = zp.tile([1, 512], bf16, name="zib")
                    nc.vector.tensor_copy(zib, zi)
                    zbs = zp.tile([64, 512], bf16, name="zbs")
                    nc.gpsimd.partition_broadcast(zbs, zib)
                    nc.vector.tensor_tensor(
                        ctxp[rows, p, 512 * lh : 512 * (lh + 1)],
                        cps[0:HD, :],
                        zbs,
                        mult,
                    )

                hp, halves = pend
                cps_last = {}
                cps_last[0] = ctx_ps.tile([128, 512], f32, name="cps", tag="cps")
                for jt in range(NLT):
                    attnv_part(hp, halves, 0, jt, cps_last[0])
                attnv_finish_lh(hp, 0, cps_last[0])
                cps_last[1] = ctx_ps.tile([128, 512], f32, name="cps", tag="cps")
                for jt in range(NLT):
                    attnv_part(hp, halves, 1, jt, cps_last[1])
                attnv_finish_lh(hp, 1, cps_last[1])
                # out-proj: the lh=0 half's PE work hides the lh=1 finish
                # chain (DVE reciprocal + GPSIMD broadcast + normalize)
                for lt in range(NLT):
                    o = ost.tile([128, D], bf16, name="o")
                    for jh in range(2):
                        outproj_unit(lt, jh, o)
                    lsl = slice(128 * lt, 128 * (lt + 1))
                    nc.sync.dma_start(out=out_d[lsl, :], in_=o)

    nc.compile()
    return nc


TRACE = False
TRACE_KWARGS = {}
LAST_RESULT = None

_NC_CACHE = None


def _get_nc():
    global _NC_CACHE
    if _NC_CACHE is None:
        _NC_CACHE = _build_bass()
    return _NC_CACHE


def _shuffle_dt(x):
    """[128*n, m] -> [128, n*m] with out[p, i*m+c] = x[128*i + p, c]."""
    n = x.shape[0] // 128
    m = x.shape[1]
    return np.ascontiguousarray(
        x.reshape(n, 128, m).transpose(1, 0, 2).reshape(128, n * m)
    )


def make_in_maps(k, v, q, E, Wk, Wv, Wq, Wo):
    """Host-side sharding: returns per-core input dicts."""
    eT = np.ascontiguousarray(E[MAX_SEQ - L :, :].T)  # [64, 1024]
    e2 = np.concatenate([eT, eT], axis=0).astype(BF16)  # [128, 1024]
    slab = (
        (np.arange(640)[None, :] - 512) <= np.arange(128)[:, None]
    ).astype(BF16)
    qkvT = {}
    for b in range(B):
        qkvT[b] = (
            _shuffle_dt(np.asarray(q[b]).T.astype(BF16)),
            _shuffle_dt(np.asarray(k[b]).T.astype(BF16)),
            _shuffle_dt(np.asarray(v[b]).T.astype(BF16)),
        )
    in_maps = []
    for core in range(NCORES):
        b, hg = divmod(core, 2)
        csl = slice(DG * hg, DG * (hg + 1))
        qTb, kTb, vTb = qkvT[b]
        in_maps.append(
            {
                "qT": qTb,
                "kT": kTb,
                "vT": vTb,
                "wq": _shuffle_dt(Wq[:, csl].astype(BF16)),
                "wk": _shuffle_dt(Wk[:, csl].astype(BF16)),
                "wv": _shuffle_dt(Wv[:, csl].astype(BF16)),
                "wo": _shuffle_dt(
                    Wo[DG * hg : DG * (hg + 1), :].astype(BF16)
                ),
                "e2": e2,
                "slab": slab,
            }
        )
    return in_maps


def kernel(
    k,
    v,
    q,
    mask,
    E,
    Wk,
    bk,
    Wv,
    bv,
    Wq,
    bq,
    Wo,
    bo,
):
    k = np.asarray(k, np.float32)
    v = np.asarray(v, np.float32)
    q = np.asarray(q, np.float32)
    E = np.asarray(E, np.float32)
    Wk = np.asarray(Wk, np.float32)
    Wv = np.asarray(Wv, np.float32)
    Wq = np.asarray(Wq, np.float32)
    Wo = np.asarray(Wo, np.float32)
    mask = np.asarray(mask)
    assert bool(mask.all()), "kernel specialized for all-true mask"
    for bias in (bk, bv, bq):
        assert not np.any(np.asarray(bias)), "kernel specialized for zero qkv biases"
    bo = np.asarray(bo, np.float32)

    from concourse.bass_utils import run_bass_kernel_spmd

    nc = _get_nc()
    in_maps = make_in_maps(k, v, q, E, Wk, Wv, Wq, Wo)
    res = run_bass_kernel_spmd(
        nc, in_maps, core_ids=list(range(NCORES)), trace=TRACE, **TRACE_KWARGS
    )
    global LAST_RESULT
    LAST_RESULT = res
    out = np.zeros((B, L, D), np.float32)
    for core in range(NCORES):
        b = core // 2
        out[b] += np.asarray(res.results[core]["out"], np.float32)
    out += bo[None, None, :]
    return out


# revision 63
# speedup vs baseline: 1.1934x; 1.1934x over previous
"""Trainium2 Bass kernel for nn_MultiHeadedAttention_51737176047655.

Multi-head attention with Music-Transformer relative position bias
(skew trick), B=4, L=1024, D=1024, 16 heads, head_dim=64.

Sharding (8 cores): core = 2*b + hg  -> batch b in [0,4), head-group hg in
[0,2).  Each core computes 8 heads for one batch over the full sequence:
  - Wq/Wk/Wv column-sharded [1024, 512], Wo row-sharded [512, 1024]
  - per-core output is a partial [1024, 1024]; host sums the two
    head-group partials per batch (standard TP unshard) and adds bo.

Device algorithm per core (matmuls bf16 in / f32 PSUM accumulate):
  qT/kT/vT arrive host-transposed [d, l]; projections give qhT/khT
  [d', l] (transposed) and vh [l, d'] (natural, with a ones column per
  head for softmax sums).  QE = qh e^T is computed per head (only the
  m < l0+128 span that survives the tri mask), masked via a shifted-tri
  "slab" multiply, and written to a DRAM scratch in the padded layout
  (row stride 1025); reading rows back with stride 1024 materializes
  the skewed Srel exactly (the reference's pad+reshape trick).  scores
  are computed TRANSPOSED (scores^T = kh qh^T, head pairs packed into
  PE row groups via tile_position) and Srel^T is accumulated into the
  same PSUM bank by transpose-by-identity matmuls, skipping
  statically-zero 128x128 blocks; exp via ScalarE (scale=1/8) ->
  unnormalized attn^T (bf16); ctx^T_aug = [vh|1]^T attn^T per head
  (row 64 = softmax denominators Z); 1/Z via a single ScalarE
  Reciprocal activation straight off the PSUM Z row, broadcast across
  64 partitions with a K=1 PE matmul, and applied by the DVE while
  packing ctx^T head-pairs; out = ctx Wo, emitted bf16 (host
  accumulates in f32).
  The TensorE instruction stream is interleaved at j-tile granularity
  (scores of head h / attnV of head h-1 / QE of head h+2, and attnV of
  the last head with the first half of the output projection) so the
  in-order PE never stalls on the exp/normalize pipelines; this keeps
  the PE clock at its top p-state.  DMA count is minimized: batched
  input loads, one sliding-window batched read for the low-half Srel
  stripes, persistent pre-zeroed stripe staging tiles.
No max-subtraction in softmax: logits are ~N(0, 1.4^2), far inside
fp32/exp range (validated vs reference at ~1e-6 in fp32 emulation).
"""

import math
import sys

import numpy as np

sys.path.insert(0, "/opt/trn_rl_repo")

import ml_dtypes  # noqa: E402

BF16 = ml_dtypes.bfloat16

# Problem constants (hardcoded per contract)
B = 4
L = 1024
D = 1024
H = 16
HD = 64
H_LOC = 8  # heads per core
DG = 512  # d' columns per core (H_LOC * HD)
NCORES = 8
MAX_SEQ = 2048
PAD = L + 1  # 1025, padded row stride of the skew scratch
FLAT = L * PAD  # 1049600 elements per head scratch

NLT = L // 128  # 8 l-tiles
NDT = D // 128  # 8 contraction tiles
NPAIR = H_LOC // 2  # 4 head pairs


def _build_bass():
    """Build the single-core SPMD Bass program (same program, per-core data)."""
    import concourse.bass as bass
    import concourse.tile as tile
    from concourse import bacc, mybir
    from concourse.dve_ops import (
        RECIP_APPROX_FAST_CONSTS,
        RECIPROCAL_APPROX_FAST,
    )

    f32 = mybir.dt.float32
    bf16 = mybir.dt.bfloat16
    Exp = mybir.ActivationFunctionType.Exp
    mult = mybir.AluOpType.mult

    nc = bacc.Bacc(
        "TRN2", target_bir_lowering=False, debug=False, enable_asserts=False
    )

    # ---- kernel I/O; qT/kT/vT are host-transposed [d, l] and then
    # host-shuffled to [p, dt, l] so device loads are contiguous ----
    qT_d = nc.declare_dram_parameter("qT", [128, NDT * L], bf16, isOutput=False)
    kT_d = nc.declare_dram_parameter("kT", [128, NDT * L], bf16, isOutput=False)
    vT_d = nc.declare_dram_parameter("vT", [128, NDT * L], bf16, isOutput=False)
    wq_d = nc.declare_dram_parameter("wq", [128, NDT * DG], bf16, isOutput=False)
    wk_d = nc.declare_dram_parameter("wk", [128, NDT * DG], bf16, isOutput=False)
    wv_d = nc.declare_dram_parameter("wv", [128, NDT * DG], bf16, isOutput=False)
    wo_d = nc.declare_dram_parameter("wo", [128, NPAIR * D], bf16, isOutput=False)
    e2_d = nc.declare_dram_parameter("e2", [128, L], bf16, isOutput=False)
    slab_d = nc.declare_dram_parameter("slab", [128, 640], bf16, isOutput=False)
    out_d = nc.declare_dram_parameter("out", [L, D], bf16, isOutput=True)

    # skew scratch, one padded buffer per local head
    scratch = [nc.dram_tensor(f"skew{h}", [FLAT], bf16) for h in range(H_LOC)]

    # block (lt, jt) of Srel is identically zero unless piece A
    # (j <= 2l-1023) or piece B (l+2 <= j <= 2l+3) intersects it.
    def srel_block_nonzero(lt, jt):
        l1 = 128 * lt + 127
        j0, j1 = 128 * jt, 128 * jt + 127
        a = 2 * l1 - 1023 >= j0
        b = (j1 >= 128 * lt + 2) and (j0 <= 2 * l1 + 3)
        return a or b

    def srel_block_lspan(lt, jt):
        """Block-relative [lo, hi) of l rows where Srel(block) is nonzero."""
        l0, l1 = 128 * lt, 128 * lt + 127
        j0, j1 = 128 * jt, 128 * jt + 127
        spans = []
        bs, be = max(l0, -(-(j0 - 3) // 2)), min(l1, j1 - 2)
        if bs <= be:
            spans.append((bs, be))
        as_, ae = max(l0, -(-(j0 + 1023) // 2)), l1
        if as_ <= ae:
            spans.append((as_, ae))
        if not spans:
            return None
        lo = min(s for s, _ in spans) - l0
        hi = max(e for _, e in spans) - l0 + 1
        return lo, hi

    with tile.TileContext(nc) as tc:
        from contextlib import ExitStack

        with ExitStack() as outer:
            # ---------------- persistent pools ----------------
            persist = outer.enter_context(tc.tile_pool(name="persist", bufs=1))
            # projection outputs (live through whole kernel)
            qhT = persist.tile([128, NPAIR, L], bf16)  # [part, pair, l]
            khT = persist.tile([128, NPAIR, L], bf16)
            # vh with ones column per head: [part(j%128), jt, head, 65]
            vh = persist.tile([128, NLT, H_LOC, HD + 1], bf16)
            e2_sb = persist.tile([128, L], bf16)
            slab_sb = persist.tile([128, 640], bf16)
            ctxp = persist.tile([128, NPAIR, L], bf16)  # packed ctx^T per pair
            # all-ones stationary for the K=1 1/Z PE broadcast
            ones1 = persist.tile([1, 64], bf16, name="ones1")
            # persistent stripe staging tiles, pre-zeroed once; data spans are
            # rewritten per head, zero-col/tail spans stay zero across reuse
            stripes = [
                [persist.tile([128, 4, PAD], bf16, name=f"st{lh}{par}")
                 for par in range(2)]
                for lh in range(2)
            ]

            nc.vector.memset(vh[:, :, :, HD : HD + 1], 1.0)
            nc.vector.memset(ones1, 1.0)
            for lh in range(2):
                for par in range(2):
                    nc.gpsimd.memset(stripes[lh][par], 0.0)

            # ---------------- phase 1+2: loads + projections ----
            with ExitStack() as outer2:
                sc_ps = outer2.enter_context(
                    tc.tile_pool(name="sc_ps", bufs=5, space="PSUM")
                )
                qe_ps = sc_ps  # QE shares the scores PSUM slots (tag "sc")
                ctx_ps = None  # opened after mm_ps closes (PSUM bank budget)
                attT = outer2.enter_context(tc.tile_pool(name="attT", bufs=4))
                srl = outer2.enter_context(tc.tile_pool(name="srl", bufs=3))
                zp = outer2.enter_context(tc.tile_pool(name="zp", bufs=2))

                ident = persist.tile([128, 128], bf16, name="ident")
                from concourse.masks import make_identity

                make_identity(nc, ident)

                # short-lived input pools opened last (LIFO close order)
                tin_blk = ExitStack()
                tin = tin_blk.enter_context(tc.tile_pool(name="tin", bufs=1))
                mm_ps = tin_blk.enter_context(
                    tc.tile_pool(name="mm_ps", bufs=3, space="PSUM")
                )
                # NOTE: phase-1 PSUM = mm(3) + sc(5) = 8 banks

                # qT and vT share one slot (vT loads after q-proj drains qT)
                qT = tin.tile([128, NDT, L], bf16, name="qT", tag="xqv")
                kT = tin.tile([128, NDT, L], bf16, name="kT")
                vT = tin.tile([128, NDT, L], bf16, name="vT", tag="xqv")
                wq_sb = tin.tile([128, NDT, DG], bf16, name="wq")
                wk_sb = tin.tile([128, NDT, DG], bf16, name="wk")
                wv_sb = tin.tile([128, NDT, DG], bf16, name="wv")

                def load_xT(dst, src_d, eng):
                    # 2-tile chunks, contiguous per partition in DRAM,
                    # incremental tile availability
                    for c in range(0, NDT, 2):
                        eng.dma_start(
                            out=dst[:, c : c + 2, :],
                            in_=src_d[:, c * L : (c + 2) * L],
                        )

                def load_w(dst, src_d, eng):
                    for c in range(0, NDT, 2):
                        eng.dma_start(
                            out=dst[:, c : c + 2, :],
                            in_=src_d[:, c * DG : (c + 2) * DG],
                        )

                # q + Wq first so the q projections start early; the input
                # wire is ~160GB/s regardless of queue count, so keep all
                # loads ordered on the sync queue
                load_w(wq_sb, wq_d, nc.sync)
                load_xT(qT, qT_d, nc.sync)
                nc.sync.dma_start(out=e2_sb, in_=e2_d[:, :])
                nc.sync.dma_start(out=slab_sb, in_=slab_d[:, :])
                load_w(wk_sb, wk_d, nc.sync)
                load_xT(kT, kT_d, nc.sync)
                load_w(wv_sb, wv_d, nc.sync)

                def proj_pair(w_sb, xT, dst, p):
                    for lh in range(2):
                        ps = mm_ps.tile([128, 512], f32, name="proj_ps", tag="mm")
                        lsl = slice(512 * lh, 512 * (lh + 1))
                        for dt in range(NDT):
                            nc.tensor.matmul(
                                ps,
                                w_sb[:, dt, 128 * p : 128 * (p + 1)],
                                xT[:, dt, lsl],
                                start=(dt == 0),
                                stop=(dt == NDT - 1),
                            )
                        nc.scalar.copy(dst[:, p, lsl], ps)

                def vh_tile(jt):
                    ps = mm_ps.tile([128, 512], f32, name="vh_ps", tag="mm")
                    jsl = slice(128 * jt, 128 * (jt + 1))
                    for dt in range(NDT):
                        nc.tensor.matmul(
                            ps,
                            vT[:, dt, jsl],
                            wv_sb[:, dt, :],
                            start=(dt == 0),
                            stop=(dt == NDT - 1),
                        )
                    # scatter 512 d' columns into per-head [64] slots with one
                    # strided copy (dst strides over the 65-wide head slots)
                    pv = ps[0:128, :]
                    ps3 = bass.AP(
                        tensor=pv.tensor,
                        offset=pv.offset,
                        ap=[list(pv.ap)[0], [HD, H_LOC], [1, HD]],
                    )
                    nc.scalar.copy(vh[:, jt, :, 0:HD], ps3)

                def qe_lt_mm(h, lt):
                    """QE matmuls (half-array config) for row-block lt;
                    returns psum tiles for qe_lt_fix."""
                    p, hl = divmod(h, 2)
                    rows = slice(64 * hl, 64 * (hl + 1))
                    tp = (64 * hl, 0)
                    l0 = 128 * lt
                    lsl = slice(l0, l0 + 128)
                    if lt <= 3:
                        n0 = l0 + 128
                        psm = qe_ps.tile([128, 512], f32, name="qe", tag="sc")
                        nc.tensor.matmul(
                            psm[:, 0:n0],
                            qhT[rows, p, lsl],
                            e2_sb[rows, 0:n0],
                            start=True,
                            stop=True,
                            tile_position=tp,
                        )
                        return (psm, None)
                    psm = qe_ps.tile([128, 512], f32, name="qe", tag="sc")
                    nc.tensor.matmul(
                        psm,
                        qhT[rows, p, lsl],
                        e2_sb[rows, 0:512],
                        start=True,
                        stop=True,
                        tile_position=tp,
                    )
                    n1 = l0 + 128 - 512
                    psm2 = qe_ps.tile([128, 512], f32, name="qe", tag="sc")
                    nc.tensor.matmul(
                        psm2[:, 0:n1],
                        qhT[rows, p, lsl],
                        e2_sb[rows, 512 : 512 + n1],
                        start=True,
                        stop=True,
                        tile_position=tp,
                    )
                    return (psm, psm2)

                def qe_lt_fix(h, lt, pss):
                    """DVE masking + stripe staging + (on the 4th block of an
                    l-half) the stripe DMA."""
                    psm, psm2 = pss
                    lh, a = divmod(lt, 4)
                    big = stripes[lh][h % 2]
                    l0 = 128 * lt
                    stripe = big[:, a, :]
                    if lt <= 3:
                        n0 = l0 + 128
                        nc.vector.tensor_tensor(
                            stripe[:, 1 : 1 + n0],
                            psm[:, 0:n0],
                            slab_sb[:, 512 - l0 : 640],
                            mult,
                        )
                    else:
                        n1 = l0 + 128 - 512
                        if lt == 4:
                            nc.vector.tensor_tensor(
                                stripe[:, 1:513],
                                psm,
                                slab_sb[:, 0:512],
                                mult,
                            )
                        else:
                            # m < 512 is fully below the diagonal: copy
                            nc.vector.tensor_copy(stripe[:, 1:513], psm)
                        nc.vector.tensor_tensor(
                            stripe[:, 513 : 1 + l0 + 128],
                            psm2[:, 0:n1],
                            slab_sb[:, 1024 - l0 : 640],
                            mult,
                        )
                    if a == 3:
                        # one DMA for the 4 padded stripes of this l-half
                        dst = bass.AP(
                            tensor=scratch[h],
                            offset=512 * lh * PAD,
                            ap=[[PAD, 128], [128 * PAD, 4], [1, PAD]],
                        )
                        nc.sync.dma_start(out=dst, in_=big)

                def qe_lt(h, lt):
                    qe_lt_fix(h, lt, qe_lt_mm(h, lt))

                def srel_load(h, lh):
                    if lh == 0:
                        # low l-half: sliding 640-wide j-window per lt
                        # (window start 128*lt covers all nonzero blocks)
                        srel = srl.tile([128, 4, 640], bf16, name="srel")
                        src = bass.AP(
                            tensor=scratch[h],
                            offset=L,
                            ap=[[L, 128], [128 * L + 128, 4], [1, 640]],
                        )
                    else:
                        # high l-half: dense
                        srel = srl.tile([128, 4, L], bf16, name="srel")
                        src = bass.AP(
                            tensor=scratch[h],
                            offset=(512 * lh + 1) * L,
                            ap=[[L, 128], [128 * L, 4], [1, L]],
                        )
                    nc.sync.dma_start(out=srel, in_=src)
                    return srel

                def scores_mm(h, lh, jt):
                    """scores^T matmul (half-array config); returns psum."""
                    p, hl = divmod(h, 2)
                    rows = slice(64 * hl, 64 * (hl + 1))
                    tp = (64 * hl, 0)
                    lsl = slice(512 * lh, 512 * (lh + 1))
                    jsl = slice(128 * jt, 128 * (jt + 1))
                    ps = sc_ps.tile([128, 512], f32, name="sc", tag="sc")
                    nzs = [
                        a for a in range(4)
                        if srel_block_nonzero(4 * lh + a, jt)
                    ]
                    nc.tensor.matmul(
                        ps,
                        khT[rows, p, jsl],
                        qhT[rows, p, lsl],
                        start=True,
                        stop=(len(nzs) == 0),
                        tile_position=tp,
                    )
                    return ps

                def scores_fix(h, lh, jt, ps, srel, att):
                    """Srel^T transpose-adds (full-array config) + exp."""
                    jsl = slice(128 * jt, 128 * (jt + 1))
                    nzs = [
                        a for a in range(4)
                        if srel_block_nonzero(4 * lh + a, jt)
                    ]
                    for i, a in enumerate(nzs):
                        if lh == 0:
                            jr = slice(128 * (jt - a), 128 * (jt - a) + 128)
                            chunk = srel[:, a, jr]
                        else:
                            chunk = srel[:, a, jsl]
                        nc.tensor.matmul(
                            ps[:, 128 * a : 128 * a + 128],
                            chunk,
                            ident,
                            start=False,
                            stop=(i == len(nzs) - 1),
                        )
                    nc.scalar.activation(att[:, jt, :], ps, Exp, scale=0.125)

                def attnv_part(h, halves, lh, jt, cps):
                    nc.tensor.matmul(
                        cps[0 : HD + 1, :],
                        vh[:, jt, h, :],
                        halves[lh][:, jt, :],
                        start=(jt == 0),
                        stop=(jt == NLT - 1),
                    )

                def attnv_finish_a(cps0, zpack):
                    # stage the lh=0 Z row; the real finish happens in _b
                    nc.vector.tensor_copy(zpack[0:1, :], cps0[HD : HD + 1, :])

                def attnv_finish_b(h, cps_both, zpack):
                    p, hl = divmod(h, 2)
                    rows = slice(64 * hl, 64 * (hl + 1))
                    nc.vector.tensor_copy(
                        zpack[32:33, :], cps_both[1][HD : HD + 1, :]
                    )
                    # one fast approximate DVE reciprocal (~51 ULP, 5x faster
                    # than InstReciprocal) covers both Z rows (0 and 32; the
                    # rows between hold garbage and are never read)
                    zinv = zp.tile([33, 512], f32, name="zinv")
                    nc.vector._custom_dve(
                        RECIPROCAL_APPROX_FAST,
                        out=zinv,
                        in0=zpack,
                        **RECIP_APPROX_FAST_CONSTS,
                    )
                    zinvb = [
                        zp.tile([1, 512], bf16, name="zinvb") for _ in range(2)
                    ]
                    for lh in range(2):
                        nc.vector.tensor_copy(
                            zinvb[lh], zinv[32 * lh : 32 * lh + 1, :]
                        )
                    for lh in range(2):
                        # broadcast 1/Z across 64 partitions on the (idle)
                        # GPSIMD engine -- no PSUM, no PE involvement
                        zbs = zp.tile([64, 512], bf16, name="zbs")
                        nc.gpsimd.partition_broadcast(zbs, zinvb[lh])
                        # normalize + pack into head-pair ctx^T (bf16)
                        nc.vector.tensor_tensor(
                            ctxp[rows, p, 512 * lh : 512 * (lh + 1)],
                            cps_both[lh][0:HD, :],
                            zbs,
                            mult,
                        )

                # ---- emission: projections first ----
                for p in range(NPAIR):
                    proj_pair(wq_sb, qT, qhT, p)
                # vT reuses qT's slot; its DMA fires once q-proj drains qT
                load_xT(vT, vT_d, nc.sync)
                for lt in range(8):
                    qe_lt(0, lt)
                for lt in range(8):
                    qe_lt(1, lt)
                for p in range(NPAIR):
                    proj_pair(wk_sb, kT, khT, p)
                for jt in range(NLT):
                    vh_tile(jt)
                tin_blk.close()
                ctx_ps = outer2.enter_context(
                    tc.tile_pool(name="ctx_ps", bufs=3, space="PSUM")
                )
                # wo lives in the space vacated by the input tiles; loaded
                # here (well before the output projection)
                wop = outer2.enter_context(tc.tile_pool(name="wop", bufs=1))
                wo_sb = wop.tile([128, NPAIR, D], bf16, name="wo")
                nc.sync.dma_start(out=wo_sb, in_=wo_d[:, :])
                ost = outer2.enter_context(tc.tile_pool(name="ost", bufs=4))

                # ---- main pipeline: scores(h) / attnV(h-1) / QE(h+2)
                # interleaved at j-tile granularity so the in-order PE
                # always has a ready instruction ----
                pend = None
                fin = None  # deferred finish_b of head h-2
                srel_cur = srel_load(0, 0)
                for h in range(H_LOC):
                    att0 = attT.tile([128, NLT, 512], bf16, name="attnT")
                    att1 = attT.tile([128, NLT, 512], bf16, name="attnT")
                    # prefetch this head's high-half Srel during the low half
                    srel_nxt = srel_load(h, 1)
                    cps_prev = {}
                    zpack = zp.tile([33, 512], f32, name="zpack")
                    for lh in range(2):
                        att = att0 if lh == 0 else att1
                        if lh == 1:
                            srel_cur = srel_nxt
                            if h + 1 < H_LOC:
                                # prefetch the next head's low half
                                srel_nxt = srel_load(h + 1, 0)
                        if pend is not None:
                            hp, halves = pend
                            cps_prev[lh] = ctx_ps.tile(
                                [128, 512], f32, name="cps", tag="cps"
                            )
                        for jt in range(NLT):
                            if pend is not None:
                                attnv_part(hp, halves, lh, jt, cps_prev[lh])
                            ps = scores_mm(h, lh, jt)
                            scores_fix(h, lh, jt, ps, srel_cur, att)
                            if lh == 0 and jt == 2 and fin is not None:
                                attnv_finish_b(*fin)
                                fin = None
                            if jt % 2 == 1 and h + 2 < H_LOC:
                                qe_lt(h + 2, 4 * lh + jt // 2)
                        if lh == 0 and pend is not None:
                            attnv_finish_a(cps_prev[0], zpack)
                    if pend is not None:
                        # defer the finish (normalize chain) into the next
                        # head's stream so the head boundary never stalls
                        fin = (hp, cps_prev, zpack)
                    pend = (h, [att0, att1])
                    srel_cur = srel_nxt
                if fin is not None:
                    attnv_finish_b(*fin)
                    fin = None

                # ---- tail: attnV of the last head, interleaved with the
                # first half of the output projection ----
                def outproj_unit(lt, jh, o):
                    lsl = slice(128 * lt, 128 * (lt + 1))
                    jsl = slice(512 * jh, 512 * (jh + 1))
                    ps = sc_ps.tile([128, 512], f32, name="op", tag="sc")
                    for p in range(NPAIR):
                        nc.tensor.matmul(
                            ps,
                            ctxp[:, p, lsl],
                            wo_sb[:, p, jsl],
                            start=(p == 0),
                            stop=(p == NPAIR - 1),
                        )
                    nc.scalar.copy(o[:, jsl], ps)

                def attnv_finish_lh(h, lh, cps):
                    # per-half finish used in the tail (DVE has slack there)
                    p, hl = divmod(h, 2)
                    rows = slice(64 * hl, 64 * (hl + 1))
                    zs = zp.tile([1, 512], f32, name="zs")
                    nc.vector.tensor_copy(zs, cps[HD : HD + 1, :])
                    zi = zp.tile([1, 512], f32, name="zi")
                    nc.vector._custom_dve(
                        RECIPROCAL_APPROX_FAST,
                        out=zi,
                        in0=zs,
                        **RECIP_APPROX_FAST_CONSTS,
                    )
                    zib = zp.tile([1, 512], bf16, name="zib")
                    nc.vector.tensor_copy(zib, zi)
                    zbs = zp.tile([64, 512], bf16, name="zbs")
                    nc.gpsimd.partition_broadcast(zbs, zib)
                    nc.vector.tensor_tensor(
                        ctxp[rows, p, 512 * lh : 512 * (lh + 1)],
                        cps[0:HD, :],
                        zbs,
                        mult,
                    )

                hp, halves = pend
                cps_last = {}
                cps_last[0] = ctx_ps.tile([128, 512], f32, name="cps", tag="cps")
                for jt in range(NLT):
                    attnv_part(hp, halves, 0, jt, cps_last[0])
                attnv_finish_lh(hp, 0, cps_last[0])
                cps_last[1] = ctx_ps.tile([128, 512], f32, name="cps", tag="cps")
                for jt in range(NLT):
                    attnv_part(hp, halves, 1, jt, cps_last[1])
                attnv_finish_lh(hp, 1, cps_last[1])
                # out-proj: the lh=0 half's PE work hides the lh=1 finish
                # chain (DVE reciprocal + GPSIMD broadcast + normalize)
                for lt in range(NLT):
                    o = ost.tile([128, D], bf16, name="o")
                    for jh in range(2):
                        outproj_unit(lt, jh, o)
                    lsl = slice(128 * lt, 128 * (lt + 1))
                    nc.sync.dma_start(out=out_d[lsl, :], in_=o)

    nc.compile()
    return nc


TRACE = False
TRACE_KWARGS = {}
LAST_RESULT = None

_NC_CACHE = None


def _get_nc():
    global _NC_CACHE
    if _NC_CACHE is None:
        _NC_CACHE = _build_bass()
    return _NC_CACHE


def _shuffle_dt(x):
    """[128*n, m] -> [128, n*m] with out[p, i*m+c] = x[128*i + p, c]."""
    n = x.shape[0] // 128
    m = x.shape[1]
    return np.ascontiguousarray(
        x.reshape(n, 128, m).transpose(1, 0, 2).reshape(128, n * m)
    )


def make_in_maps(k, v, q, E, Wk, Wv, Wq, Wo):
    """Host-side sharding: returns per-core input dicts."""
    eT = np.ascontiguousarray(E[MAX_SEQ - L :, :].T)  # [64, 1024]
    e2 = np.concatenate([eT, eT], axis=0).astype(BF16)  # [128, 1024]
    slab = (
        (np.arange(640)[None, :] - 512) <= np.arange(128)[:, None]
    ).astype(BF16)
    qkvT = {}
    for b in range(B):
        qkvT[b] = (
            _shuffle_dt(np.asarray(q[b]).T.astype(BF16)),
            _shuffle_dt(np.asarray(k[b]).T.astype(BF16)),
            _shuffle_dt(np.asarray(v[b]).T.astype(BF16)),
        )
    in_maps = []
    for core in range(NCORES):
        b, hg = divmod(core, 2)
        csl = slice(DG * hg, DG * (hg + 1))
        qTb, kTb, vTb = qkvT[b]
        in_maps.append(
            {
                "qT": qTb,
                "kT": kTb,
                "vT": vTb,
                "wq": _shuffle_dt(Wq[:, csl].astype(BF16)),
                "wk": _shuffle_dt(Wk[:, csl].astype(BF16)),
                "wv": _shuffle_dt(Wv[:, csl].astype(BF16)),
                "wo": _shuffle_dt(
                    Wo[DG * hg : DG * (hg + 1), :].astype(BF16)
                ),
                "e2": e2,
                "slab": slab,
            }
        )
    return in_maps


def kernel(
    k,
    v,
    q,
    mask,
    E,
    Wk,
    bk,
    Wv,
    bv,
    Wq,
    bq,
    Wo,
    bo,
):
    k = np.asarray(k, np.float32)
    v = np.asarray(v, np.float32)
    q = np.asarray(q, np.float32)
    E = np.asarray(E, np.float32)
    Wk = np.asarray(Wk, np.float32)
    Wv = np.asarray(Wv, np.float32)
    Wq = np.asarray(Wq, np.float32)
    Wo = np.asarray(Wo, np.float32)
    mask = np.asarray(mask)
    assert bool(mask.all()), "kernel specialized for all-true mask"
    for bias in (bk, bv, bq):
        assert not np.any(np.asarray(bias)), "kernel specialized for zero qkv biases"
    bo = np.asarray(bo, np.float32)

    from concourse.bass_utils import run_bass_kernel_spmd

    nc = _get_nc()
    in_maps = make_in_maps(k, v, q, E, Wk, Wv, Wq, Wo)
    res = run_bass_kernel_spmd(
        nc, in_maps, core_ids=list(range(NCORES)), trace=TRACE, **TRACE_KWARGS
    )
    global LAST_RESULT
    LAST_RESULT = res
    out = np.zeros((B, L, D), np.float32)
    for core in range(NCORES):
        b = core // 2
        out[b] += np.asarray(res.results[core]["out"], np.float32)
    out += bo[None, None, :]
    return out


# revision 64
# speedup vs baseline: 1.1983x; 1.0040x over previous
"""Trainium2 Bass kernel for nn_MultiHeadedAttention_51737176047655.

Multi-head attention with Music-Transformer relative position bias
(skew trick), B=4, L=1024, D=1024, 16 heads, head_dim=64.

Sharding (8 cores): core = 2*b + hg  -> batch b in [0,4), head-group hg in
[0,2).  Each core computes 8 heads for one batch over the full sequence:
  - Wq/Wk/Wv column-sharded [1024, 512], Wo row-sharded [512, 1024]
  - per-core output is a partial [1024, 1024]; host sums the two
    head-group partials per batch (standard TP unshard) and adds bo.

Device algorithm per core (matmuls bf16 in / f32 PSUM accumulate):
  qT/kT/vT arrive host-transposed [d, l]; projections give qhT/khT
  [d', l] (transposed) and vh [l, d'] (natural, with a ones column per
  head for softmax sums).  QE = qh e^T is computed per head (only the
  m < l0+128 span that survives the tri mask), masked via a shifted-tri
  "slab" multiply, and written to a DRAM scratch in the padded layout
  (row stride 1025); reading rows back with stride 1024 materializes
  the skewed Srel exactly (the reference's pad+reshape trick).  scores
  are computed TRANSPOSED (scores^T = kh qh^T, head pairs packed into
  PE row groups via tile_position) and Srel^T is accumulated into the
  same PSUM bank by transpose-by-identity matmuls, skipping
  statically-zero 128x128 blocks; exp via ScalarE (scale=1/8) ->
  unnormalized attn^T (bf16); ctx^T_aug = [vh|1]^T attn^T per head
  (row 64 = softmax denominators Z); 1/Z via a single ScalarE
  Reciprocal activation straight off the PSUM Z row, broadcast across
  64 partitions with a K=1 PE matmul, and applied by the DVE while
  packing ctx^T head-pairs; out = ctx Wo, emitted bf16 (host
  accumulates in f32).
  The TensorE instruction stream is interleaved at j-tile granularity
  (scores of head h / attnV of head h-1 / QE of head h+2, and attnV of
  the last head with the first half of the output projection) so the
  in-order PE never stalls on the exp/normalize pipelines; this keeps
  the PE clock at its top p-state.  DMA count is minimized: batched
  input loads, one sliding-window batched read for the low-half Srel
  stripes, persistent pre-zeroed stripe staging tiles.
No max-subtraction in softmax: logits are ~N(0, 1.4^2), far inside
fp32/exp range (validated vs reference at ~1e-6 in fp32 emulation).
"""

import math
import sys

import numpy as np

sys.path.insert(0, "/opt/trn_rl_repo")

import ml_dtypes  # noqa: E402

BF16 = ml_dtypes.bfloat16

# Problem constants (hardcoded per contract)
B = 4
L = 1024
D = 1024
H = 16
HD = 64
H_LOC = 8  # heads per core
DG = 512  # d' columns per core (H_LOC * HD)
NCORES = 8
MAX_SEQ = 2048
PAD = L + 1  # 1025, padded row stride of the skew scratch
FLAT = L * PAD  # 1049600 elements per head scratch

NLT = L // 128  # 8 l-tiles
NDT = D // 128  # 8 contraction tiles
NPAIR = H_LOC // 2  # 4 head pairs


def _build_bass():
    """Build the single-core SPMD Bass program (same program, per-core data)."""
    import concourse.bass as bass
    import concourse.tile as tile
    from concourse import bacc, mybir
    from concourse.dve_ops import (
        RECIP_APPROX_FAST_CONSTS,
        RECIPROCAL_APPROX_FAST,
    )

    f32 = mybir.dt.float32
    bf16 = mybir.dt.bfloat16
    Exp = mybir.ActivationFunctionType.Exp
    mult = mybir.AluOpType.mult

    nc = bacc.Bacc(
        "TRN2", target_bir_lowering=False, debug=False, enable_asserts=False
    )

    # ---- kernel I/O; qT/kT/vT are host-transposed [d, l] and then
    # host-shuffled to [p, dt, l] so device loads are contiguous ----
    qT_d = nc.declare_dram_parameter("qT", [128, NDT * L], bf16, isOutput=False)
    kT_d = nc.declare_dram_parameter("kT", [128, NDT * L], bf16, isOutput=False)
    vT_d = nc.declare_dram_parameter("vT", [128, NDT * L], bf16, isOutput=False)
    wq_d = nc.declare_dram_parameter("wq", [128, NDT * DG], bf16, isOutput=False)
    wk_d = nc.declare_dram_parameter("wk", [128, NDT * DG], bf16, isOutput=False)
    wv_d = nc.declare_dram_parameter("wv", [128, NDT * DG], bf16, isOutput=False)
    wo_d = nc.declare_dram_parameter("wo", [128, NPAIR * D], bf16, isOutput=False)
    e2_d = nc.declare_dram_parameter("e2", [128, L], bf16, isOutput=False)
    slab_d = nc.declare_dram_parameter("slab", [128, 640], bf16, isOutput=False)
    out_d = nc.declare_dram_parameter("out", [L, D], bf16, isOutput=True)

    # skew scratch, one padded buffer per local head
    scratch = [nc.dram_tensor(f"skew{h}", [FLAT], bf16) for h in range(H_LOC)]

    # block (lt, jt) of Srel is identically zero unless piece A
    # (j <= 2l-1023) or piece B (l+2 <= j <= 2l+3) intersects it.
    def srel_block_nonzero(lt, jt):
        l1 = 128 * lt + 127
        j0, j1 = 128 * jt, 128 * jt + 127
        a = 2 * l1 - 1023 >= j0
        b = (j1 >= 128 * lt + 2) and (j0 <= 2 * l1 + 3)
        return a or b

    def srel_block_lspan(lt, jt):
        """Block-relative [lo, hi) of l rows where Srel(block) is nonzero."""
        l0, l1 = 128 * lt, 128 * lt + 127
        j0, j1 = 128 * jt, 128 * jt + 127
        spans = []
        bs, be = max(l0, -(-(j0 - 3) // 2)), min(l1, j1 - 2)
        if bs <= be:
            spans.append((bs, be))
        as_, ae = max(l0, -(-(j0 + 1023) // 2)), l1
        if as_ <= ae:
            spans.append((as_, ae))
        if not spans:
            return None
        lo = min(s for s, _ in spans) - l0
        hi = max(e for _, e in spans) - l0 + 1
        return lo, hi

    with tile.TileContext(nc) as tc:
        from contextlib import ExitStack

        with ExitStack() as outer:
            # ---------------- persistent pools ----------------
            persist = outer.enter_context(tc.tile_pool(name="persist", bufs=1))
            # projection outputs (live through whole kernel)
            qhT = persist.tile([128, NPAIR, L], bf16)  # [part, pair, l]
            khT = persist.tile([128, NPAIR, L], bf16)
            # vh with ones column per head: [part(j%128), jt, head, 65]
            vh = persist.tile([128, NLT, H_LOC, HD + 1], bf16)
            e2_sb = persist.tile([128, L], bf16)
            slab_sb = persist.tile([128, 640], bf16)
            ctxp = persist.tile([128, NPAIR, L], bf16)  # packed ctx^T per pair
            # all-ones stationary for the K=1 1/Z PE broadcast
            ones1 = persist.tile([1, 64], bf16, name="ones1")
            # persistent stripe staging tiles, pre-zeroed once; data spans are
            # rewritten per head, zero-col/tail spans stay zero across reuse
            stripes = [
                [persist.tile([128, 4, PAD], bf16, name=f"st{lh}{par}")
                 for par in range(2)]
                for lh in range(2)
            ]

            nc.vector.memset(vh[:, :, :, HD : HD + 1], 1.0)
            nc.vector.memset(ones1, 1.0)
            for lh in range(2):
                for par in range(2):
                    nc.gpsimd.memset(stripes[lh][par], 0.0)

            # ---------------- phase 1+2: loads + projections ----
            with ExitStack() as outer2:
                sc_ps = outer2.enter_context(
                    tc.tile_pool(name="sc_ps", bufs=5, space="PSUM")
                )
                qe_ps = sc_ps  # QE shares the scores PSUM slots (tag "sc")
                ctx_ps = None  # opened after mm_ps closes (PSUM bank budget)
                attT = outer2.enter_context(tc.tile_pool(name="attT", bufs=4))
                srl = outer2.enter_context(tc.tile_pool(name="srl", bufs=3))
                zp = outer2.enter_context(tc.tile_pool(name="zp", bufs=2))

                ident = persist.tile([128, 128], bf16, name="ident")
                from concourse.masks import make_identity

                make_identity(nc, ident)

                # short-lived input pools opened last (LIFO close order)
                tin_blk = ExitStack()
                tin = tin_blk.enter_context(tc.tile_pool(name="tin", bufs=1))
                mm_ps = tin_blk.enter_context(
                    tc.tile_pool(name="mm_ps", bufs=3, space="PSUM")
                )
                # NOTE: phase-1 PSUM = mm(3) + sc(5) = 8 banks

                # qT and vT share one slot (vT loads after q-proj drains qT)
                qT = tin.tile([128, NDT, L], bf16, name="qT", tag="xqv")
                kT = tin.tile([128, NDT, L], bf16, name="kT")
                vT = tin.tile([128, NDT, L], bf16, name="vT", tag="xqv")
                wq_sb = tin.tile([128, NDT, DG], bf16, name="wq")
                wk_sb = tin.tile([128, NDT, DG], bf16, name="wk")
                wv_sb = tin.tile([128, NDT, DG], bf16, name="wv")

                def load_xT(dst, src_d, eng):
                    # 2-tile chunks, contiguous per partition in DRAM,
                    # incremental tile availability
                    for c in range(0, NDT, 2):
                        eng.dma_start(
                            out=dst[:, c : c + 2, :],
                            in_=src_d[:, c * L : (c + 2) * L],
                        )

                def load_w(dst, src_d, eng):
                    for c in range(0, NDT, 2):
                        eng.dma_start(
                            out=dst[:, c : c + 2, :],
                            in_=src_d[:, c * DG : (c + 2) * DG],
                        )

                # q + Wq first so the q projections start early; the input
                # wire is ~160GB/s regardless of queue count, so keep all
                # loads ordered on the sync queue
                load_w(wq_sb, wq_d, nc.sync)
                load_xT(qT, qT_d, nc.sync)
                nc.sync.dma_start(out=e2_sb, in_=e2_d[:, :])
                nc.sync.dma_start(out=slab_sb, in_=slab_d[:, :])
                load_w(wk_sb, wk_d, nc.sync)
                load_xT(kT, kT_d, nc.sync)
                load_w(wv_sb, wv_d, nc.sync)

                def proj_pair(w_sb, xT, dst, p):
                    for lh in range(2):
                        ps = mm_ps.tile([128, 512], f32, name="proj_ps", tag="mm")
                        lsl = slice(512 * lh, 512 * (lh + 1))
                        for dt in range(NDT):
                            nc.tensor.matmul(
                                ps,
                                w_sb[:, dt, 128 * p : 128 * (p + 1)],
                                xT[:, dt, lsl],
                                start=(dt == 0),
                                stop=(dt == NDT - 1),
                            )
                        nc.scalar.copy(dst[:, p, lsl], ps)

                def vh_tile(jt):
                    ps = mm_ps.tile([128, 512], f32, name="vh_ps", tag="mm")
                    jsl = slice(128 * jt, 128 * (jt + 1))
                    for dt in range(NDT):
                        nc.tensor.matmul(
                            ps,
                            vT[:, dt, jsl],
                            wv_sb[:, dt, :],
                            start=(dt == 0),
                            stop=(dt == NDT - 1),
                        )
                    # scatter 512 d' columns into per-head [64] slots with one
                    # strided copy (dst strides over the 65-wide head slots)
                    pv = ps[0:128, :]
                    ps3 = bass.AP(
                        tensor=pv.tensor,
                        offset=pv.offset,
                        ap=[list(pv.ap)[0], [HD, H_LOC], [1, HD]],
                    )
                    nc.scalar.copy(vh[:, jt, :, 0:HD], ps3)

                def qe_lt_mm(h, lt):
                    """QE matmuls (half-array config) for row-block lt;
                    returns psum tiles for qe_lt_fix."""
                    p, hl = divmod(h, 2)
                    rows = slice(64 * hl, 64 * (hl + 1))
                    tp = (64 * hl, 0)
                    l0 = 128 * lt
                    lsl = slice(l0, l0 + 128)
                    if lt <= 3:
                        n0 = l0 + 128
                        psm = qe_ps.tile([128, 512], f32, name="qe", tag="sc")
                        nc.tensor.matmul(
                            psm[:, 0:n0],
                            qhT[rows, p, lsl],
                            e2_sb[rows, 0:n0],
                            start=True,
                            stop=True,
                            tile_position=tp,
                        )
                        return (psm, None)
                    psm = qe_ps.tile([128, 512], f32, name="qe", tag="sc")
                    nc.tensor.matmul(
                        psm,
                        qhT[rows, p, lsl],
                        e2_sb[rows, 0:512],
                        start=True,
                        stop=True,
                        tile_position=tp,
                    )
                    n1 = l0 + 128 - 512
                    psm2 = qe_ps.tile([128, 512], f32, name="qe", tag="sc")
                    nc.tensor.matmul(
                        psm2[:, 0:n1],
                        qhT[rows, p, lsl],
                        e2_sb[rows, 512 : 512 + n1],
                        start=True,
                        stop=True,
                        tile_position=tp,
                    )
                    return (psm, psm2)

                def qe_lt_fix(h, lt, pss):
                    """DVE masking + stripe staging + (on the 4th block of an
                    l-half) the stripe DMA."""
                    psm, psm2 = pss
                    lh, a = divmod(lt, 4)
                    big = stripes[lh][h % 2]
                    l0 = 128 * lt
                    stripe = big[:, a, :]
                    if lt <= 3:
                        n0 = l0 + 128
                        nc.vector.tensor_tensor(
                            stripe[:, 1 : 1 + n0],
                            psm[:, 0:n0],
                            slab_sb[:, 512 - l0 : 640],
                            mult,
                        )
                    else:
                        n1 = l0 + 128 - 512
                        if lt == 4:
                            nc.vector.tensor_tensor(
                                stripe[:, 1:513],
                                psm,
                                slab_sb[:, 0:512],
                                mult,
                            )
                        else:
                            # m < 512 is fully below the diagonal: copy
                            nc.vector.tensor_copy(stripe[:, 1:513], psm)
                        nc.vector.tensor_tensor(
                            stripe[:, 513 : 1 + l0 + 128],
                            psm2[:, 0:n1],
                            slab_sb[:, 1024 - l0 : 640],
                            mult,
                        )
                    if a == 3:
                        # one DMA for the 4 padded stripes of this l-half
                        dst = bass.AP(
                            tensor=scratch[h],
                            offset=512 * lh * PAD,
                            ap=[[PAD, 128], [128 * PAD, 4], [1, PAD]],
                        )
                        nc.sync.dma_start(out=dst, in_=big)

                def qe_lt(h, lt):
                    qe_lt_fix(h, lt, qe_lt_mm(h, lt))

                def srel_load(h, lh):
                    if lh == 0:
                        # low l-half: sliding 640-wide j-window per lt
                        # (window start 128*lt covers all nonzero blocks)
                        srel = srl.tile([128, 4, 640], bf16, name="srel")
                        src = bass.AP(
                            tensor=scratch[h],
                            offset=L,
                            ap=[[L, 128], [128 * L + 128, 4], [1, 640]],
                        )
                    else:
                        # high l-half: dense
                        srel = srl.tile([128, 4, L], bf16, name="srel")
                        src = bass.AP(
                            tensor=scratch[h],
                            offset=(512 * lh + 1) * L,
                            ap=[[L, 128], [128 * L, 4], [1, L]],
                        )
                    nc.sync.dma_start(out=srel, in_=src)
                    return srel

                def scores_mm(h, lh, jt):
                    """scores^T matmul (half-array config); returns psum."""
                    p, hl = divmod(h, 2)
                    rows = slice(64 * hl, 64 * (hl + 1))
                    tp = (64 * hl, 0)
                    lsl = slice(512 * lh, 512 * (lh + 1))
                    jsl = slice(128 * jt, 128 * (jt + 1))
                    ps = sc_ps.tile([128, 512], f32, name="sc", tag="sc")
                    nzs = [
                        a for a in range(4)
                        if srel_block_nonzero(4 * lh + a, jt)
                    ]
                    nc.tensor.matmul(
                        ps,
                        khT[rows, p, jsl],
                        qhT[rows, p, lsl],
                        start=True,
                        stop=(len(nzs) == 0),
                        tile_position=tp,
                    )
                    return ps

                def scores_fix(h, lh, jt, ps, srel, att):
                    """Srel^T transpose-adds (full-array config) + exp."""
                    jsl = slice(128 * jt, 128 * (jt + 1))
                    nzs = [
                        a for a in range(4)
                        if srel_block_nonzero(4 * lh + a, jt)
                    ]
                    for i, a in enumerate(nzs):
                        if lh == 0:
                            jr = slice(128 * (jt - a), 128 * (jt - a) + 128)
                            chunk = srel[:, a, jr]
                        else:
                            chunk = srel[:, a, jsl]
                        nc.tensor.matmul(
                            ps[:, 128 * a : 128 * a + 128],
                            chunk,
                            ident,
                            start=False,
                            stop=(i == len(nzs) - 1),
                        )
                    nc.scalar.activation(att[:, jt, :], ps, Exp, scale=0.125)

                def attnv_part(h, halves, lh, jt, cps):
                    nc.tensor.matmul(
                        cps[0 : HD + 1, :],
                        vh[:, jt, h, :],
                        halves[lh][:, jt, :],
                        start=(jt == 0),
                        stop=(jt == NLT - 1),
                    )

                def attnv_finish_a(cps0, zpack):
                    # stage the lh=0 Z row; the real finish happens in _b
                    nc.vector.tensor_copy(zpack[0:1, :], cps0[HD : HD + 1, :])

                def attnv_finish_b(h, cps_both, zpack):
                    p, hl = divmod(h, 2)
                    rows = slice(64 * hl, 64 * (hl + 1))
                    nc.vector.tensor_copy(
                        zpack[32:33, :], cps_both[1][HD : HD + 1, :]
                    )
                    # one fast approximate DVE reciprocal (~51 ULP, 5x faster
                    # than InstReciprocal) covers both Z rows (0 and 32; the
                    # rows between hold garbage and are never read)
                    zinv = zp.tile([33, 512], f32, name="zinv")
                    nc.vector._custom_dve(
                        RECIPROCAL_APPROX_FAST,
                        out=zinv,
                        in0=zpack,
                        **RECIP_APPROX_FAST_CONSTS,
                    )
                    zinvb = [
                        zp.tile([1, 512], bf16, name="zinvb") for _ in range(2)
                    ]
                    for lh in range(2):
                        nc.vector.tensor_copy(
                            zinvb[lh], zinv[32 * lh : 32 * lh + 1, :]
                        )
                    for lh in range(2):
                        # broadcast 1/Z across 64 partitions on the (idle)
                        # GPSIMD engine -- no PSUM, no PE involvement
                        zbs = zp.tile([64, 512], bf16, name="zbs")
                        nc.gpsimd.partition_broadcast(zbs, zinvb[lh])
                        # normalize + pack into head-pair ctx^T (bf16)
                        nc.vector.tensor_tensor(
                            ctxp[rows, p, 512 * lh : 512 * (lh + 1)],
                            cps_both[lh][0:HD, :],
                            zbs,
                            mult,
                        )

                # ---- emission: projections first ----
                for p in range(NPAIR):
                    proj_pair(wq_sb, qT, qhT, p)
                # vT reuses qT's slot; its DMA fires once q-proj drains qT
                load_xT(vT, vT_d, nc.sync)
                for lt in range(8):
                    qe_lt(0, lt)
                for lt in range(8):
                    qe_lt(1, lt)
                for p in range(NPAIR):
                    proj_pair(wk_sb, kT, khT, p)
                for jt in range(NLT):
                    vh_tile(jt)
                tin_blk.close()
                ctx_ps = outer2.enter_context(
                    tc.tile_pool(name="ctx_ps", bufs=3, space="PSUM")
                )
                # wo lives in the space vacated by the input tiles; loaded
                # here (well before the output projection)
                wop = outer2.enter_context(tc.tile_pool(name="wop", bufs=1))
                wo_sb = wop.tile([128, NPAIR, D], bf16, name="wo")
                nc.sync.dma_start(out=wo_sb, in_=wo_d[:, :])
                ost = outer2.enter_context(tc.tile_pool(name="ost", bufs=4))

                # ---- main pipeline: scores(h) / attnV(h-1) / QE(h+2)
                # interleaved at j-tile granularity so the in-order PE
                # always has a ready instruction ----
                pend = None
                fin = None  # deferred finish_b of head h-2
                srel_cur = srel_load(0, 0)
                for h in range(H_LOC):
                    att0 = attT.tile([128, NLT, 512], bf16, name="attnT")
                    att1 = attT.tile([128, NLT, 512], bf16, name="attnT")
                    # prefetch this head's high-half Srel during the low half
                    srel_nxt = srel_load(h, 1)
                    cps_prev = {}
                    zpack = zp.tile([33, 512], f32, name="zpack")
                    for lh in range(2):
                        att = att0 if lh == 0 else att1
                        if lh == 1:
                            srel_cur = srel_nxt
                            if h + 1 < H_LOC:
                                # prefetch the next head's low half
                                srel_nxt = srel_load(h + 1, 0)
                        if pend is not None:
                            hp, halves = pend
                            cps_prev[lh] = ctx_ps.tile(
                                [128, 512], f32, name="cps", tag="cps"
                            )
                        for jt in range(NLT):
                            if pend is not None:
                                attnv_part(hp, halves, lh, jt, cps_prev[lh])
                            ps = scores_mm(h, lh, jt)
                            # QE matmuls ride the same half-array config as
                            # the scores matmul (array reconfig costs ~100ns)
                            qe_due = jt % 2 == 1 and h + 2 < H_LOC
                            if qe_due:
                                lt = 4 * lh + jt // 2
                                pss = qe_lt_mm(h + 2, lt)
                            scores_fix(h, lh, jt, ps, srel_cur, att)
                            if qe_due:
                                qe_lt_fix(h + 2, lt, pss)
                            if lh == 0 and jt == 2 and fin is not None:
                                attnv_finish_b(*fin)
                                fin = None
                        if lh == 0 and pend is not None:
                            attnv_finish_a(cps_prev[0], zpack)
                    if pend is not None:
                        # defer the finish (normalize chain) into the next
                        # head's stream so the head boundary never stalls
                        fin = (hp, cps_prev, zpack)
                    pend = (h, [att0, att1])
                    srel_cur = srel_nxt
                if fin is not None:
                    attnv_finish_b(*fin)
                    fin = None

                # ---- tail: attnV of the last head, interleaved with the
                # first half of the output projection ----
                def outproj_unit(lt, jh, o):
                    lsl = slice(128 * lt, 128 * (lt + 1))
                    jsl = slice(512 * jh, 512 * (jh + 1))
                    ps = sc_ps.tile([128, 512], f32, name="op", tag="sc")
                    for p in range(NPAIR):
                        nc.tensor.matmul(
                            ps,
                            ctxp[:, p, lsl],
                            wo_sb[:, p, jsl],
                            start=(p == 0),
                            stop=(p == NPAIR - 1),
                        )
                    nc.scalar.copy(o[:, jsl], ps)

                def attnv_finish_lh(h, lh, cps):
                    # per-half finish used in the tail (DVE has slack there)
                    p, hl = divmod(h, 2)
                    rows = slice(64 * hl, 64 * (hl + 1))
                    zs = zp.tile([1, 512], f32, name="zs")
                    nc.vector.tensor_copy(zs, cps[HD : HD + 1, :])
                    zi = zp.tile([1, 512], f32, name="zi")
                    nc.vector._custom_dve(
                        RECIPROCAL_APPROX_FAST,
                        out=zi,
                        in0=zs,
                        **RECIP_APPROX_FAST_CONSTS,
                    )
                    zib = zp.tile([1, 512], bf16, name="zib")
                    nc.vector.tensor_copy(zib, zi)
                    zbs = zp.tile([64, 512], bf16, name="zbs")
                    nc.gpsimd.partition_broadcast(zbs, zib)
                    nc.vector.tensor_tensor(
                        ctxp[rows, p, 512 * lh : 512 * (lh + 1)],
                        cps[0:HD, :],
                        zbs,
                        mult,
                    )

                hp, halves = pend
                cps_last = {}
                cps_last[0] = ctx_ps.tile([128, 512], f32, name="cps", tag="cps")
                for jt in range(NLT):
                    attnv_part(hp, halves, 0, jt, cps_last[0])
                attnv_finish_lh(hp, 0, cps_last[0])
                cps_last[1] = ctx_ps.tile([128, 512], f32, name="cps", tag="cps")
                for jt in range(NLT):
                    attnv_part(hp, halves, 1, jt, cps_last[1])
                attnv_finish_lh(hp, 1, cps_last[1])
                # out-proj: the lh=0 half's PE work hides the lh=1 finish
                # chain (DVE reciprocal + GPSIMD broadcast + normalize)
                for lt in range(NLT):
                    o = ost.tile([128, D], bf16, name="o")
                    for jh in range(2):
                        outproj_unit(lt, jh, o)
                    lsl = slice(128 * lt, 128 * (lt + 1))
                    nc.sync.dma_start(out=out_d[lsl, :], in_=o)

    nc.compile()
    return nc


TRACE = False
TRACE_KWARGS = {}
LAST_RESULT = None

_NC_CACHE = None


def _get_nc():
    global _NC_CACHE
    if _NC_CACHE is None:
        _NC_CACHE = _build_bass()
    return _NC_CACHE


def _shuffle_dt(x):
    """[128*n, m] -> [128, n*m] with out[p, i*m+c] = x[128*i + p, c]."""
    n = x.shape[0] // 128
    m = x.shape[1]
    return np.ascontiguousarray(
        x.reshape(n, 128, m).transpose(1, 0, 2).reshape(128, n * m)
    )


def make_in_maps(k, v, q, E, Wk, Wv, Wq, Wo):
    """Host-side sharding: returns per-core input dicts."""
    eT = np.ascontiguousarray(E[MAX_SEQ - L :, :].T)  # [64, 1024]
    e2 = np.concatenate([eT, eT], axis=0).astype(BF16)  # [128, 1024]
    slab = (
        (np.arange(640)[None, :] - 512) <= np.arange(128)[:, None]
    ).astype(BF16)
    qkvT = {}
    for b in range(B):
        qkvT[b] = (
            _shuffle_dt(np.asarray(q[b]).T.astype(BF16)),
            _shuffle_dt(np.asarray(k[b]).T.astype(BF16)),
            _shuffle_dt(np.asarray(v[b]).T.astype(BF16)),
        )
    in_maps = []
    for core in range(NCORES):
        b, hg = divmod(core, 2)
        csl = slice(DG * hg, DG * (hg + 1))
        qTb, kTb, vTb = qkvT[b]
        in_maps.append(
            {
                "qT": qTb,
                "kT": kTb,
                "vT": vTb,
                "wq": _shuffle_dt(Wq[:, csl].astype(BF16)),
                "wk": _shuffle_dt(Wk[:, csl].astype(BF16)),
                "wv": _shuffle_dt(Wv[:, csl].astype(BF16)),
                "wo": _shuffle_dt(
                    Wo[DG * hg : DG * (hg + 1), :].astype(BF16)
                ),
                "e2": e2,
                "slab": slab,
            }
        )
    return in_maps


def kernel(
    k,
    v,
    q,
    mask,
    E,
    Wk,
    bk,
    Wv,
    bv,
    Wq,
    bq,
    Wo,
    bo,
):
    k = np.asarray(k, np.float32)
    v = np.asarray(v, np.float32)
    q = np.asarray(q, np.float32)
    E = np.asarray(E, np.float32)
    Wk = np.asarray(Wk, np.float32)
    Wv = np.asarray(Wv, np.float32)
    Wq = np.asarray(Wq, np.float32)
    Wo = np.asarray(Wo, np.float32)
    mask = np.asarray(mask)
    assert bool(mask.all()), "kernel specialized for all-true mask"
    for bias in (bk, bv, bq):
        assert not np.any(np.asarray(bias)), "kernel specialized for zero qkv biases"
    bo = np.asarray(bo, np.float32)

    from concourse.bass_utils import run_bass_kernel_spmd

    nc = _get_nc()
    in_maps = make_in_maps(k, v, q, E, Wk, Wv, Wq, Wo)
    res = run_bass_kernel_spmd(
        nc, in_maps, core_ids=list(range(NCORES)), trace=TRACE, **TRACE_KWARGS
    )
    global LAST_RESULT
    LAST_RESULT = res
    out = np.zeros((B, L, D), np.float32)
    for core in range(NCORES):
        b = core // 2
        out[b] += np.asarray(res.results[core]["out"], np.float32)
    out += bo[None, None, :]
    return out


# revision 65
# speedup vs baseline: 1.2099x; 1.0097x over previous
"""Trainium2 Bass kernel for nn_MultiHeadedAttention_51737176047655.

Multi-head attention with Music-Transformer relative position bias
(skew trick), B=4, L=1024, D=1024, 16 heads, head_dim=64.

Sharding (8 cores): core = 2*b + hg  -> batch b in [0,4), head-group hg in
[0,2).  Each core computes 8 heads for one batch over the full sequence:
  - Wq/Wk/Wv column-sharded [1024, 512], Wo row-sharded [512, 1024]
  - per-core output is a partial [1024, 1024]; host sums the two
    head-group partials per batch (standard TP unshard) and adds bo.

Device algorithm per core (matmuls bf16 in / f32 PSUM accumulate):
  qT/kT/vT arrive host-transposed [d, l]; projections give qhT/khT
  [d', l] (transposed) and vh [l, d'] (natural, with a ones column per
  head for softmax sums).  QE = qh e^T is computed per head (only the
  m < l0+128 span that survives the tri mask), masked via a shifted-tri
  "slab" multiply, and written to a DRAM scratch in the padded layout
  (row stride 1025); reading rows back with stride 1024 materializes
  the skewed Srel exactly (the reference's pad+reshape trick).  scores
  are computed TRANSPOSED (scores^T = kh qh^T, head pairs packed into
  PE row groups via tile_position) and Srel^T is accumulated into the
  same PSUM bank by transpose-by-identity matmuls, skipping
  statically-zero 128x128 blocks; exp via ScalarE (scale=1/8) ->
  unnormalized attn^T (bf16); ctx^T_aug = [vh|1]^T attn^T per head
  (row 64 = softmax denominators Z); 1/Z via a single ScalarE
  Reciprocal activation straight off the PSUM Z row, broadcast across
  64 partitions with a K=1 PE matmul, and applied by the DVE while
  packing ctx^T head-pairs; out = ctx Wo, emitted bf16 (host
  accumulates in f32).
  The TensorE instruction stream is interleaved at j-tile granularity
  (scores of head h / attnV of head h-1 / QE of head h+2, and attnV of
  the last head with the first half of the output projection) so the
  in-order PE never stalls on the exp/normalize pipelines; this keeps
  the PE clock at its top p-state.  DMA count is minimized: batched
  input loads, one sliding-window batched read for the low-half Srel
  stripes, persistent pre-zeroed stripe staging tiles.
No max-subtraction in softmax: logits are ~N(0, 1.4^2), far inside
fp32/exp range (validated vs reference at ~1e-6 in fp32 emulation).
"""

import math
import sys

import numpy as np

sys.path.insert(0, "/opt/trn_rl_repo")

import ml_dtypes  # noqa: E402

BF16 = ml_dtypes.bfloat16

# Problem constants (hardcoded per contract)
B = 4
L = 1024
D = 1024
H = 16
HD = 64
H_LOC = 8  # heads per core
DG = 512  # d' columns per core (H_LOC * HD)
NCORES = 8
MAX_SEQ = 2048
PAD = L + 1  # 1025, padded row stride of the skew scratch
FLAT = L * PAD  # 1049600 elements per head scratch

NLT = L // 128  # 8 l-tiles
NDT = D // 128  # 8 contraction tiles
NPAIR = H_LOC // 2  # 4 head pairs


def _build_bass():
    """Build the single-core SPMD Bass program (same program, per-core data)."""
    import concourse.bass as bass
    import concourse.tile as tile
    from concourse import bacc, mybir
    from concourse.dve_ops import (
        RECIP_APPROX_FAST_CONSTS,
        RECIPROCAL_APPROX_FAST,
    )

    f32 = mybir.dt.float32
    bf16 = mybir.dt.bfloat16
    Exp = mybir.ActivationFunctionType.Exp
    mult = mybir.AluOpType.mult

    nc = bacc.Bacc(
        "TRN2", target_bir_lowering=False, debug=False, enable_asserts=False
    )

    # ---- kernel I/O; qT/kT/vT are host-transposed [d, l] and then
    # host-shuffled to [p, dt, l] so device loads are contiguous ----
    qT_d = nc.declare_dram_parameter("qT", [128, NDT * L], bf16, isOutput=False)
    kT_d = nc.declare_dram_parameter("kT", [128, NDT * L], bf16, isOutput=False)
    vT_d = nc.declare_dram_parameter("vT", [128, NDT * L], bf16, isOutput=False)
    wq_d = nc.declare_dram_parameter("wq", [128, NDT * DG], bf16, isOutput=False)
    wk_d = nc.declare_dram_parameter("wk", [128, NDT * DG], bf16, isOutput=False)
    wv_d = nc.declare_dram_parameter("wv", [128, NDT * DG], bf16, isOutput=False)
    wo_d = nc.declare_dram_parameter("wo", [128, NPAIR * D], bf16, isOutput=False)
    e2_d = nc.declare_dram_parameter("e2", [128, L], bf16, isOutput=False)
    slab_d = nc.declare_dram_parameter("slab", [128, 640], bf16, isOutput=False)
    out_d = nc.declare_dram_parameter("out", [L, D], bf16, isOutput=True)

    # skew scratch, one padded buffer per local head
    scratch = [nc.dram_tensor(f"skew{h}", [FLAT], bf16) for h in range(H_LOC)]

    # block (lt, jt) of Srel is identically zero unless piece A
    # (j <= 2l-1023) or piece B (l+2 <= j <= 2l+3) intersects it.
    def srel_block_nonzero(lt, jt):
        l1 = 128 * lt + 127
        j0, j1 = 128 * jt, 128 * jt + 127
        a = 2 * l1 - 1023 >= j0
        b = (j1 >= 128 * lt + 2) and (j0 <= 2 * l1 + 3)
        return a or b

    def srel_block_lspan(lt, jt):
        """Block-relative [lo, hi) of l rows where Srel(block) is nonzero."""
        l0, l1 = 128 * lt, 128 * lt + 127
        j0, j1 = 128 * jt, 128 * jt + 127
        spans = []
        bs, be = max(l0, -(-(j0 - 3) // 2)), min(l1, j1 - 2)
        if bs <= be:
            spans.append((bs, be))
        as_, ae = max(l0, -(-(j0 + 1023) // 2)), l1
        if as_ <= ae:
            spans.append((as_, ae))
        if not spans:
            return None
        lo = min(s for s, _ in spans) - l0
        hi = max(e for _, e in spans) - l0 + 1
        return lo, hi

    with tile.TileContext(nc) as tc:
        from contextlib import ExitStack

        with ExitStack() as outer:
            # ---------------- persistent pools ----------------
            persist = outer.enter_context(tc.tile_pool(name="persist", bufs=1))
            # projection outputs (live through whole kernel)
            qhT = persist.tile([128, NPAIR, L], bf16)  # [part, pair, l]
            khT = persist.tile([128, NPAIR, L], bf16)
            # vh with ones column per head: [part(j%128), jt, head, 65]
            vh = persist.tile([128, NLT, H_LOC, HD + 1], bf16)
            e2_sb = persist.tile([128, L], bf16)
            slab_sb = persist.tile([128, 640], bf16)
            ctxp = persist.tile([128, NPAIR, L], bf16)  # packed ctx^T per pair
            # all-ones stationary for the K=1 1/Z PE broadcast
            ones1 = persist.tile([1, 64], bf16, name="ones1")
            # persistent stripe staging tiles, pre-zeroed once; data spans are
            # rewritten per head, zero-col/tail spans stay zero across reuse
            stripes = [
                [persist.tile([128, 4, PAD], bf16, name=f"st{lh}{par}")
                 for par in range(2)]
                for lh in range(2)
            ]

            nc.vector.memset(vh[:, :, :, HD : HD + 1], 1.0)
            nc.vector.memset(ones1, 1.0)
            for lh in range(2):
                for par in range(2):
                    nc.gpsimd.memset(stripes[lh][par], 0.0)

            # ---------------- phase 1+2: loads + projections ----
            with ExitStack() as outer2:
                sc_ps = outer2.enter_context(
                    tc.tile_pool(name="sc_ps", bufs=5, space="PSUM")
                )
                qe_ps = sc_ps  # QE shares the scores PSUM slots (tag "sc")
                ctx_ps = None  # opened after mm_ps closes (PSUM bank budget)
                attT = outer2.enter_context(tc.tile_pool(name="attT", bufs=4))
                srl = outer2.enter_context(tc.tile_pool(name="srl", bufs=3))
                zp = outer2.enter_context(tc.tile_pool(name="zp", bufs=2))

                ident = persist.tile([128, 128], bf16, name="ident")
                from concourse.masks import make_identity

                make_identity(nc, ident)

                # short-lived input pools opened last (LIFO close order)
                tin_blk = ExitStack()
                tin = tin_blk.enter_context(tc.tile_pool(name="tin", bufs=1))
                mm_ps = tin_blk.enter_context(
                    tc.tile_pool(name="mm_ps", bufs=3, space="PSUM")
                )
                # NOTE: phase-1 PSUM = mm(3) + sc(5) = 8 banks

                # qT and vT share one slot (vT loads after q-proj drains qT)
                qT = tin.tile([128, NDT, L], bf16, name="qT", tag="xqv")
                kT = tin.tile([128, NDT, L], bf16, name="kT")
                vT = tin.tile([128, NDT, L], bf16, name="vT", tag="xqv")
                wq_sb = tin.tile([128, NDT, DG], bf16, name="wq")
                wk_sb = tin.tile([128, NDT, DG], bf16, name="wk")
                wv_sb = tin.tile([128, NDT, DG], bf16, name="wv")

                def load_xT(dst, src_d, eng):
                    # 2-tile chunks, contiguous per partition in DRAM,
                    # incremental tile availability
                    for c in range(0, NDT, 2):
                        eng.dma_start(
                            out=dst[:, c : c + 2, :],
                            in_=src_d[:, c * L : (c + 2) * L],
                        )

                def load_w(dst, src_d, eng):
                    for c in range(0, NDT, 2):
                        eng.dma_start(
                            out=dst[:, c : c + 2, :],
                            in_=src_d[:, c * DG : (c + 2) * DG],
                        )

                # q + Wq first, chunk-interleaved in consumption order so
                # the first projection chain starts as early as possible
                for c in range(0, NDT, 2):
                    nc.sync.dma_start(
                        out=wq_sb[:, c : c + 2, :],
                        in_=wq_d[:, c * DG : (c + 2) * DG],
                    )
                    nc.sync.dma_start(
                        out=qT[:, c : c + 2, :],
                        in_=qT_d[:, c * L : (c + 2) * L],
                    )
                nc.sync.dma_start(out=e2_sb, in_=e2_d[:, :])
                nc.sync.dma_start(out=slab_sb, in_=slab_d[:, :])
                load_w(wk_sb, wk_d, nc.sync)
                load_xT(kT, kT_d, nc.sync)
                load_w(wv_sb, wv_d, nc.sync)

                def proj_pair(w_sb, xT, dst, p):
                    for lh in range(2):
                        ps = mm_ps.tile([128, 512], f32, name="proj_ps", tag="mm")
                        lsl = slice(512 * lh, 512 * (lh + 1))
                        for dt in range(NDT):
                            nc.tensor.matmul(
                                ps,
                                w_sb[:, dt, 128 * p : 128 * (p + 1)],
                                xT[:, dt, lsl],
                                start=(dt == 0),
                                stop=(dt == NDT - 1),
                            )
                        nc.scalar.copy(dst[:, p, lsl], ps)

                def vh_tile(jt):
                    ps = mm_ps.tile([128, 512], f32, name="vh_ps", tag="mm")
                    jsl = slice(128 * jt, 128 * (jt + 1))
                    for dt in range(NDT):
                        nc.tensor.matmul(
                            ps,
                            vT[:, dt, jsl],
                            wv_sb[:, dt, :],
                            start=(dt == 0),
                            stop=(dt == NDT - 1),
                        )
                    # scatter 512 d' columns into per-head [64] slots with one
                    # strided copy (dst strides over the 65-wide head slots)
                    pv = ps[0:128, :]
                    ps3 = bass.AP(
                        tensor=pv.tensor,
                        offset=pv.offset,
                        ap=[list(pv.ap)[0], [HD, H_LOC], [1, HD]],
                    )
                    nc.scalar.copy(vh[:, jt, :, 0:HD], ps3)

                def qe_lt_mm(h, lt):
                    """QE matmuls (half-array config) for row-block lt;
                    returns psum tiles for qe_lt_fix."""
                    p, hl = divmod(h, 2)
                    rows = slice(64 * hl, 64 * (hl + 1))
                    tp = (64 * hl, 0)
                    l0 = 128 * lt
                    lsl = slice(l0, l0 + 128)
                    if lt <= 3:
                        n0 = l0 + 128
                        psm = qe_ps.tile([128, 512], f32, name="qe", tag="sc")
                        nc.tensor.matmul(
                            psm[:, 0:n0],
                            qhT[rows, p, lsl],
                            e2_sb[rows, 0:n0],
                            start=True,
                            stop=True,
                            tile_position=tp,
                        )
                        return (psm, None)
                    psm = qe_ps.tile([128, 512], f32, name="qe", tag="sc")
                    nc.tensor.matmul(
                        psm,
                        qhT[rows, p, lsl],
                        e2_sb[rows, 0:512],
                        start=True,
                        stop=True,
                        tile_position=tp,
                    )
                    n1 = l0 + 128 - 512
                    psm2 = qe_ps.tile([128, 512], f32, name="qe", tag="sc")
                    nc.tensor.matmul(
                        psm2[:, 0:n1],
                        qhT[rows, p, lsl],
                        e2_sb[rows, 512 : 512 + n1],
                        start=True,
                        stop=True,
                        tile_position=tp,
                    )
                    return (psm, psm2)

                def qe_lt_fix(h, lt, pss):
                    """DVE masking + stripe staging + (on the 4th block of an
                    l-half) the stripe DMA."""
                    psm, psm2 = pss
                    lh, a = divmod(lt, 4)
                    big = stripes[lh][h % 2]
                    l0 = 128 * lt
                    stripe = big[:, a, :]
                    if lt <= 3:
                        n0 = l0 + 128
                        nc.vector.tensor_tensor(
                            stripe[:, 1 : 1 + n0],
                            psm[:, 0:n0],
                            slab_sb[:, 512 - l0 : 640],
                            mult,
                        )
                    else:
                        n1 = l0 + 128 - 512
                        if lt == 4:
                            nc.vector.tensor_tensor(
                                stripe[:, 1:513],
                                psm,
                                slab_sb[:, 0:512],
                                mult,
                            )
                        else:
                            # m < 512 is fully below the diagonal: copy
                            nc.vector.tensor_copy(stripe[:, 1:513], psm)
                        nc.vector.tensor_tensor(
                            stripe[:, 513 : 1 + l0 + 128],
                            psm2[:, 0:n1],
                            slab_sb[:, 1024 - l0 : 640],
                            mult,
                        )
                    if a == 3:
                        # one DMA for the 4 padded stripes of this l-half
                        dst = bass.AP(
                            tensor=scratch[h],
                            offset=512 * lh * PAD,
                            ap=[[PAD, 128], [128 * PAD, 4], [1, PAD]],
                        )
                        nc.sync.dma_start(out=dst, in_=big)

                def qe_lt(h, lt):
                    qe_lt_fix(h, lt, qe_lt_mm(h, lt))

                def srel_load(h, lh):
                    if lh == 0:
                        # low l-half: sliding 640-wide j-window per lt
                        # (window start 128*lt covers all nonzero blocks)
                        srel = srl.tile([128, 4, 640], bf16, name="srel")
                        src = bass.AP(
                            tensor=scratch[h],
                            offset=L,
                            ap=[[L, 128], [128 * L + 128, 4], [1, 640]],
                        )
                    else:
                        # high l-half: dense
                        srel = srl.tile([128, 4, L], bf16, name="srel")
                        src = bass.AP(
                            tensor=scratch[h],
                            offset=(512 * lh + 1) * L,
                            ap=[[L, 128], [128 * L, 4], [1, L]],
                        )
                    nc.sync.dma_start(out=srel, in_=src)
                    return srel

                def scores_mm(h, lh, jt):
                    """scores^T matmul (half-array config); returns psum."""
                    p, hl = divmod(h, 2)
                    rows = slice(64 * hl, 64 * (hl + 1))
                    tp = (64 * hl, 0)
                    lsl = slice(512 * lh, 512 * (lh + 1))
                    jsl = slice(128 * jt, 128 * (jt + 1))
                    ps = sc_ps.tile([128, 512], f32, name="sc", tag="sc")
                    nzs = [
                        a for a in range(4)
                        if srel_block_nonzero(4 * lh + a, jt)
                    ]
                    nc.tensor.matmul(
                        ps,
                        khT[rows, p, jsl],
                        qhT[rows, p, lsl],
                        start=True,
                        stop=(len(nzs) == 0),
                        tile_position=tp,
                    )
                    return ps

                def scores_fix(h, lh, jt, ps, srel, att):
                    """Srel^T transpose-adds (full-array config) + exp."""
                    jsl = slice(128 * jt, 128 * (jt + 1))
                    nzs = [
                        a for a in range(4)
                        if srel_block_nonzero(4 * lh + a, jt)
                    ]
                    for i, a in enumerate(nzs):
                        if lh == 0:
                            jr = slice(128 * (jt - a), 128 * (jt - a) + 128)
                            chunk = srel[:, a, jr]
                        else:
                            chunk = srel[:, a, jsl]
                        nc.tensor.matmul(
                            ps[:, 128 * a : 128 * a + 128],
                            chunk,
                            ident,
                            start=False,
                            stop=(i == len(nzs) - 1),
                        )
                    nc.scalar.activation(att[:, jt, :], ps, Exp, scale=0.125)

                def attnv_part(h, halves, lh, jt, cps):
                    nc.tensor.matmul(
                        cps[0 : HD + 1, :],
                        vh[:, jt, h, :],
                        halves[lh][:, jt, :],
                        start=(jt == 0),
                        stop=(jt == NLT - 1),
                    )

                def attnv_finish_a(cps0, zpack):
                    # stage the lh=0 Z row; the real finish happens in _b
                    nc.vector.tensor_copy(zpack[0:1, :], cps0[HD : HD + 1, :])

                def attnv_finish_b(h, cps_both, zpack):
                    p, hl = divmod(h, 2)
                    rows = slice(64 * hl, 64 * (hl + 1))
                    nc.vector.tensor_copy(
                        zpack[32:33, :], cps_both[1][HD : HD + 1, :]
                    )
                    # one fast approximate DVE reciprocal (~51 ULP, 5x faster
                    # than InstReciprocal) covers both Z rows (0 and 32; the
                    # rows between hold garbage and are never read)
                    zinv = zp.tile([33, 512], f32, name="zinv")
                    nc.vector._custom_dve(
                        RECIPROCAL_APPROX_FAST,
                        out=zinv,
                        in0=zpack,
                        **RECIP_APPROX_FAST_CONSTS,
                    )
                    zinvb = [
                        zp.tile([1, 512], bf16, name="zinvb") for _ in range(2)
                    ]
                    for lh in range(2):
                        nc.vector.tensor_copy(
                            zinvb[lh], zinv[32 * lh : 32 * lh + 1, :]
                        )
                    for lh in range(2):
                        # broadcast 1/Z across 64 partitions on the (idle)
                        # GPSIMD engine -- no PSUM, no PE involvement
                        zbs = zp.tile([64, 512], bf16, name="zbs")
                        nc.gpsimd.partition_broadcast(zbs, zinvb[lh])
                        # normalize + pack into head-pair ctx^T (bf16)
                        nc.vector.tensor_tensor(
                            ctxp[rows, p, 512 * lh : 512 * (lh + 1)],
                            cps_both[lh][0:HD, :],
                            zbs,
                            mult,
                        )

                # ---- emission: projections first ----
                for p in range(NPAIR):
                    proj_pair(wq_sb, qT, qhT, p)
                # vT reuses qT's slot; its DMA fires once q-proj drains qT
                load_xT(vT, vT_d, nc.sync)
                for lt in range(8):
                    qe_lt(0, lt)
                for lt in range(8):
                    qe_lt(1, lt)
                for p in range(NPAIR):
                    proj_pair(wk_sb, kT, khT, p)
                for jt in range(NLT):
                    vh_tile(jt)
                tin_blk.close()
                ctx_ps = outer2.enter_context(
                    tc.tile_pool(name="ctx_ps", bufs=3, space="PSUM")
                )
                # wo lives in the space vacated by the input tiles; loaded
                # here (well before the output projection)
                wop = outer2.enter_context(tc.tile_pool(name="wop", bufs=1))
                wo_sb = wop.tile([128, NPAIR, D], bf16, name="wo")
                nc.sync.dma_start(out=wo_sb, in_=wo_d[:, :])
                ost = outer2.enter_context(tc.tile_pool(name="ost", bufs=4))

                # ---- main pipeline: scores(h) / attnV(h-1) / QE(h+2)
                # interleaved at j-tile granularity so the in-order PE
                # always has a ready instruction ----
                pend = None
                fin = None  # deferred finish_b of head h-2
                srel_cur = srel_load(0, 0)
                for h in range(H_LOC):
                    att0 = attT.tile([128, NLT, 512], bf16, name="attnT")
                    att1 = attT.tile([128, NLT, 512], bf16, name="attnT")
                    # prefetch this head's high-half Srel during the low half
                    srel_nxt = srel_load(h, 1)
                    cps_prev = {}
                    zpack = zp.tile([33, 512], f32, name="zpack")
                    for lh in range(2):
                        att = att0 if lh == 0 else att1
                        if lh == 1:
                            srel_cur = srel_nxt
                            if h + 1 < H_LOC:
                                # prefetch the next head's low half
                                srel_nxt = srel_load(h + 1, 0)
                        if pend is not None:
                            hp, halves = pend
                            cps_prev[lh] = ctx_ps.tile(
                                [128, 512], f32, name="cps", tag="cps"
                            )
                        for jt in range(NLT):
                            if pend is not None:
                                attnv_part(hp, halves, lh, jt, cps_prev[lh])
                            ps = scores_mm(h, lh, jt)
                            # QE matmuls ride the same half-array config as
                            # the scores matmul (array reconfig costs ~100ns)
                            qe_due = jt % 2 == 1 and h + 2 < H_LOC
                            if qe_due:
                                lt = 4 * lh + jt // 2
                                pss = qe_lt_mm(h + 2, lt)
                            scores_fix(h, lh, jt, ps, srel_cur, att)
                            if qe_due:
                                qe_lt_fix(h + 2, lt, pss)
                            if lh == 0 and jt == 2 and fin is not None:
                                attnv_finish_b(*fin)
                                fin = None
                        if lh == 0 and pend is not None:
                            attnv_finish_a(cps_prev[0], zpack)
                    if pend is not None:
                        # defer the finish (normalize chain) into the next
                        # head's stream so the head boundary never stalls
                        fin = (hp, cps_prev, zpack)
                    pend = (h, [att0, att1])
                    srel_cur = srel_nxt
                if fin is not None:
                    attnv_finish_b(*fin)
                    fin = None

                # ---- tail: attnV of the last head, interleaved with the
                # first half of the output projection ----
                def outproj_unit(lt, jh, o):
                    lsl = slice(128 * lt, 128 * (lt + 1))
                    jsl = slice(512 * jh, 512 * (jh + 1))
                    ps = sc_ps.tile([128, 512], f32, name="op", tag="sc")
                    for p in range(NPAIR):
                        nc.tensor.matmul(
                            ps,
                            ctxp[:, p, lsl],
                            wo_sb[:, p, jsl],
                            start=(p == 0),
                            stop=(p == NPAIR - 1),
                        )
                    nc.scalar.copy(o[:, jsl], ps)

                def attnv_finish_lh(h, lh, cps):
                    # per-half finish used in the tail (DVE has slack there)
                    p, hl = divmod(h, 2)
                    rows = slice(64 * hl, 64 * (hl + 1))
                    zs = zp.tile([1, 512], f32, name="zs")
                    nc.vector.tensor_copy(zs, cps[HD : HD + 1, :])
                    zi = zp.tile([1, 512], f32, name="zi")
                    nc.vector._custom_dve(
                        RECIPROCAL_APPROX_FAST,
                        out=zi,
                        in0=zs,
                        **RECIP_APPROX_FAST_CONSTS,
                    )
                    zib = zp.tile([1, 512], bf16, name="zib")
                    nc.vector.tensor_copy(zib, zi)
                    zbs = zp.tile([64, 512], bf16, name="zbs")
                    nc.gpsimd.partition_broadcast(zbs, zib)
                    nc.vector.tensor_tensor(
                        ctxp[rows, p, 512 * lh : 512 * (lh + 1)],
                        cps[0:HD, :],
                        zbs,
                        mult,
                    )

                hp, halves = pend
                cps_last = {}
                cps_last[0] = ctx_ps.tile([128, 512], f32, name="cps", tag="cps")
                for jt in range(NLT):
                    attnv_part(hp, halves, 0, jt, cps_last[0])
                attnv_finish_lh(hp, 0, cps_last[0])
                cps_last[1] = ctx_ps.tile([128, 512], f32, name="cps", tag="cps")
                for jt in range(NLT):
                    attnv_part(hp, halves, 1, jt, cps_last[1])
                attnv_finish_lh(hp, 1, cps_last[1])
                # out-proj: the lh=0 half's PE work hides the lh=1 finish
                # chain (DVE reciprocal + GPSIMD broadcast + normalize)
                for lt in range(NLT):
                    o = ost.tile([128, D], bf16, name="o")
                    for jh in range(2):
                        outproj_unit(lt, jh, o)
                    lsl = slice(128 * lt, 128 * (lt + 1))
                    nc.sync.dma_start(out=out_d[lsl, :], in_=o)

    nc.compile()
    return nc


TRACE = False
TRACE_KWARGS = {}
LAST_RESULT = None

_NC_CACHE = None


def _get_nc():
    global _NC_CACHE
    if _NC_CACHE is None:
        _NC_CACHE = _build_bass()
    return _NC_CACHE


def _shuffle_dt(x):
    """[128*n, m] -> [128, n*m] with out[p, i*m+c] = x[128*i + p, c]."""
    n = x.shape[0] // 128
    m = x.shape[1]
    return np.ascontiguousarray(
        x.reshape(n, 128, m).transpose(1, 0, 2).reshape(128, n * m)
    )


def make_in_maps(k, v, q, E, Wk, Wv, Wq, Wo):
    """Host-side sharding: returns per-core input dicts."""
    eT = np.ascontiguousarray(E[MAX_SEQ - L :, :].T)  # [64, 1024]
    e2 = np.concatenate([eT, eT], axis=0).astype(BF16)  # [128, 1024]
    slab = (
        (np.arange(640)[None, :] - 512) <= np.arange(128)[:, None]
    ).astype(BF16)
    qkvT = {}
    for b in range(B):
        qkvT[b] = (
            _shuffle_dt(np.asarray(q[b]).T.astype(BF16)),
            _shuffle_dt(np.asarray(k[b]).T.astype(BF16)),
            _shuffle_dt(np.asarray(v[b]).T.astype(BF16)),
        )
    in_maps = []
    for core in range(NCORES):
        b, hg = divmod(core, 2)
        csl = slice(DG * hg, DG * (hg + 1))
        qTb, kTb, vTb = qkvT[b]
        in_maps.append(
            {
                "qT": qTb,
                "kT": kTb,
                "vT": vTb,
                "wq": _shuffle_dt(Wq[:, csl].astype(BF16)),
                "wk": _shuffle_dt(Wk[:, csl].astype(BF16)),
                "wv": _shuffle_dt(Wv[:, csl].astype(BF16)),
                "wo": _shuffle_dt(
                    Wo[DG * hg : DG * (hg + 1), :].astype(BF16)
                ),
                "e2": e2,
                "slab": slab,
            }
        )
    return in_maps


def kernel(
    k,
    v,
    q,
    mask,
    E,
    Wk,
    bk,
    Wv,
    bv,
    Wq,
    bq,
    Wo,
    bo,
):
    k = np.asarray(k, np.float32)
    v = np.asarray(v, np.float32)
    q = np.asarray(q, np.float32)
    E = np.asarray(E, np.float32)
    Wk = np.asarray(Wk, np.float32)
    Wv = np.asarray(Wv, np.float32)
    Wq = np.asarray(Wq, np.float32)
    Wo = np.asarray(Wo, np.float32)
    mask = np.asarray(mask)
    assert bool(mask.all()), "kernel specialized for all-true mask"
    for bias in (bk, bv, bq):
        assert not np.any(np.asarray(bias)), "kernel specialized for zero qkv biases"
    bo = np.asarray(bo, np.float32)

    from concourse.bass_utils import run_bass_kernel_spmd

    nc = _get_nc()
    in_maps = make_in_maps(k, v, q, E, Wk, Wv, Wq, Wo)
    res = run_bass_kernel_spmd(
        nc, in_maps, core_ids=list(range(NCORES)), trace=TRACE, **TRACE_KWARGS
    )
    global LAST_RESULT
    LAST_RESULT = res
    out = np.zeros((B, L, D), np.float32)
    for core in range(NCORES):
        b = core // 2
        out[b] += np.asarray(res.results[core]["out"], np.float32)
    out += bo[None, None, :]
    return out
